# revision 37
# baseline (speedup 1.0000x reference)
"""Trainium2 Bass kernel for LoopCoderAttention (sparse_attention).

Head-sharded tensor parallelism over 8 NeuronCores:
  core c owns query heads {2c, 2c+1} and KV head c//2.
All on-device tensors live in transposed [feature, token] layout so every
matmul contracts along the partition dim with zero on-device transposes
(except v, which needs one PE transpose per 128-tile).

v4 notes (on top of the v3 streaming/queue layout):
 - attention scores for BOTH heads of a step land in adjacent PSUM banks
   of one persistent 4-bank "ring" tile, so a single Exp ACTIVATE covers
   both heads (the ACT engine was the steady-state bottleneck at 2 calls
   per step). Global steps ping-pong bank pairs (0,1)/(2,3); the local
   pass packs both heads into ONE bank per key tile (256+256 columns)
   and accumulates pv_l into ring banks 2,3 — freeing enough PSUM for
   the wider score tiles (ring 4 + pv_g 2 + sums 1 + bcast 1 = 8 banks).
 - causal/band masks are applied with one DVE multiply per step against
   head-duplicated mask tables (MASKD2/MASKL2).
 - per-token scale broadcasts (ones outer products) serialize through a
   dedicated 1-bank "bcast" tile instead of stealing score banks.
 - a2a staging is 2 strided SWDGE DMAs per chunk-combine (gpsimd queue),
   immediately followed in the same FIFO by the collective trigger.

o_proj: a 2MB AllToAll reshards attention output from head-sharded to
token-sharded; each core then runs the full 2048-deep contraction for its
256-token slice (the "all-reduce" happens inside the matmul accumulation).
"""
import sys
sys.path.insert(0, '/opt/trn_rl_repo')
import numpy as np
import ml_dtypes
import concourse.bass as bass
import concourse.mybir as mybir
import concourse.tile as tile
from concourse import bacc
from concourse.bass_utils import run_bass_kernel_spmd

T = 2048
HID = 2048
HQ = 16
HK = 4
D = 128
WIN = 64
THETA = 10000.0
SCALE = D ** -0.5
NCORES = 8
TCH = 512                 # t-chunk (matmul free dim)
NCH = T // TCH            # 4 chunks
KT = HID // 128           # 16 k-tiles for 2048-deep contractions
ST = T // 128             # 16 s-tiles
TSL = T // NCORES         # 256-token output slice per core
MASKV = -1e9

F32 = mybir.dt.float32
BF16 = mybir.dt.bfloat16
AF = mybir.ActivationFunctionType

_CACHE = {}


def _build():
    nc = bacc.Bacc("TRN2", target_bir_lowering=False, debug=False,
                   num_devices=NCORES)
    HST = nc.dram_tensor("HST", [HID, T], BF16, kind="ExternalInput").ap()
    WQKV = nc.dram_tensor("WQKV", [HID, 512], BF16, kind="ExternalInput").ap()
    KGT = nc.dram_tensor("KGT", [D, T], BF16, kind="ExternalInput").ap()
    VG = nc.dram_tensor("VG", [T, D], BF16, kind="ExternalInput").ap()
    WO = nc.dram_tensor("WO", [HID, HID], BF16, kind="ExternalInput").ap()
    WG = nc.dram_tensor("WG", [D, 2], BF16, kind="ExternalInput").ap()
    BG = nc.dram_tensor("BG", [33, 1], F32, kind="ExternalInput").ap()
    CSF = nc.dram_tensor("CSF", [128, T], BF16, kind="ExternalInput").ap()
    SNF = nc.dram_tensor("SNF", [128, T], BF16, kind="ExternalInput").ap()
    ONES = nc.dram_tensor("ONES", [128, 1], BF16, kind="ExternalInput").ap()
    ONESR = nc.dram_tensor("ONESR", [128, 128], BF16, kind="ExternalInput").ap()
    IDN = nc.dram_tensor("IDN", [128, 128], BF16, kind="ExternalInput").ap()
    MASKD2 = nc.dram_tensor("MASKD2", [128, 256], BF16,
                            kind="ExternalInput").ap()
    MASKL2 = nc.dram_tensor("MASKL2", [128, 512], BF16,
                            kind="ExternalInput").ap()
    OUT = nc.dram_tensor("OUT", [TSL, HID], BF16, kind="ExternalOutput").ap()

    with tile.TileContext(nc) as tc:
        # pools are a strict stack: creation order is the reverse of the
        # release order at each phase boundary
        const = tc.alloc_tile_pool(name="const", bufs=1)
        dram = tc.alloc_tile_pool(name="dram", bufs=1, space="DRAM")
        aoutp = tc.alloc_tile_pool(name="aoutp", bufs=3)
        opool = tc.alloc_tile_pool(name="opool", bufs=1)
        osb = tc.alloc_tile_pool(name="osb", bufs=3)
        work = tc.alloc_tile_pool(name="work", bufs=1)
        ropet = tc.alloc_tile_pool(name="ropet", bufs=2)
        combp = tc.alloc_tile_pool(name="combp", bufs=2)
        wqkvp = tc.alloc_tile_pool(name="wqkvp", bufs=1)
        chunkp = tc.alloc_tile_pool(name="chunkp", bufs=2)
        hsp = tc.alloc_tile_pool(name="hsp", bufs=1)
        ps1 = tc.alloc_tile_pool(name="ps1", bufs=5, space="PSUM")

        # ---- phase-1 input streaming, interleaved across both HWDGE rings
        # so the first matmul can start right after the ~7us NRT preamble:
        #   scalar ring: wqkv, csf/snf halves, small consts
        #   sync ring:   hs high-token pair (small slabs first), low pair,
        #                then attention constants mid-loop
        wqkv_sb = wqkvp.tile([128, KT, 512], BF16)
        wqkv_view = WQKV.rearrange("(k p) c -> p k c", p=128)
        hst_sb = hsp.tile([128, KT, 4, TCH], BF16)
        hst_view = HST.rearrange("(k p) t -> p k t", p=128)
        nc.scalar.dma_start(out=wqkv_sb[:, 0:2, :], in_=wqkv_view[:, 0:2, :])
        nc.sync.dma_start(out=hst_sb[:, 0:1, 2:4, :],
                          in_=hst_view[:, 0:1, 1024:2048])
        nc.sync.dma_start(out=hst_sb[:, 1:2, 2:4, :],
                          in_=hst_view[:, 1:2, 1024:2048])
        nc.scalar.dma_start(out=wqkv_sb[:, 2:8, :], in_=wqkv_view[:, 2:8, :])
        for kg in range(1, 8):
            nc.sync.dma_start(out=hst_sb[:, 2 * kg:2 * kg + 2, 2:4, :],
                              in_=hst_view[:, 2 * kg:2 * kg + 2, 1024:2048])
        nc.scalar.dma_start(out=wqkv_sb[:, 8:16, :], in_=wqkv_view[:, 8:16, :])
        csf_sb = wqkvp.tile([128, T], BF16)
        snf_sb = wqkvp.tile([128, T], BF16)
        nc.scalar.dma_start(out=csf_sb[:, 1024:2048], in_=CSF[:, 1024:2048])
        nc.scalar.dma_start(out=snf_sb[:, 1024:2048], in_=SNF[:, 1024:2048])
        idn_sb = wqkvp.tile([128, 128], BF16)
        nc.scalar.dma_start(out=idn_sb[:], in_=IDN)
        wg_sb = const.tile([D, 2], BF16)
        nc.scalar.dma_start(out=wg_sb[:], in_=WG)
        bg_sb = const.tile([33, 1], F32)
        nc.scalar.dma_start(out=bg_sb[:], in_=BG)
        # low-token hs pair + low cos/sin: streamed while chunks 3/2 compute
        for kg in range(4):
            nc.sync.dma_start(out=hst_sb[:, 4 * kg:4 * kg + 4, 0:2, :],
                              in_=hst_view[:, 4 * kg:4 * kg + 4, 0:1024])
        nc.scalar.dma_start(out=csf_sb[:, 0:1024], in_=CSF[:, 0:1024])
        nc.scalar.dma_start(out=snf_sb[:, 0:1024], in_=SNF[:, 0:1024])
        # attention-phase constants (emitted mid phase-1 loop, see below)
        kgt_sb = const.tile([D, T], BF16)
        vg_sb = const.tile([128, ST, D], BF16)
        ones_sb = const.tile([128, 1], BF16)
        onesr_sb = const.tile([128, 128], BF16)
        maskd_sb = const.tile([128, 2, 128], BF16)
        maskl_sb = const.tile([128, 2, 256], BF16)

        # ---- persistent work tiles (through attention) ----
        qrot = work.tile([128, 2, T], BF16)
        krot = work.tile([128, T], BF16)
        vcur = work.tile([128, ST, D], BF16)   # current v in [s, d] tiles
        # gates staged at the partitions where the softmax-sum rows land:
        # row 0 = g_h0, 32 = g_h1, 64 = 1-g_h0, 96 = 1-g_h1 (per chunk n)
        gstack = work.tile([128, NCH, TCH], F32)
        SMR = (0, 32, 64, 96)
        # negated gate bias for the exp-based sigmoid of the flushed chunk
        bgn_sb = work.tile([33, 1], F32)
        nc.vector.tensor_scalar(bgn_sb[:], bg_sb[:], -1.0, 0.0,
                                mybir.AluOpType.mult, mybir.AluOpType.add)

        # tiny dummy collective fired at kernel start: absorbs the cc-stream
        # init barrier and the ~11.5us first-trigger penalty under phase 1,
        # so the real all-to-alls start promptly
        dmy_i = dram.tile([NCORES, 16], BF16)
        dmy_o = dram.tile([NCORES, NCORES, 16], BF16)
        a2ai_hi = dram.tile([NCORES, 2 * D, TSL // 2], BF16)
        a2ao_hi = dram.tile([NCORES, 2 * D, TSL // 2], BF16)
        a2ai_lo = dram.tile([NCORES, 2 * D, TSL // 2], BF16)
        a2ao_lo = dram.tile([NCORES, 2 * D, TSL // 2], BF16)

        def rope_chunk(dst_full, src, n):
            """dst_full[:, n*TCH:...] = neox-rope of chunk tile src [128, TCH].

            rot = src * [cos;cos] + rot90(src) * [-sin;sin], where rot90 swaps
            the two 64-partition halves (built with two SBUF->SBUF DMAs since
            DVE ops require matching base partitions).
            """
            sl = bass.ds(n * TCH, TCH)
            sr = ropet.tile([128, TCH], BF16, tag="ropesr", name=f"sr{n}")
            # scalar HWDGE ring: chains right behind the ACT copy that
            # produced src, and keeps the sync ring free for bulk loads
            nc.scalar.dma_start(out=sr[0:64, :], in_=src[64:128, :])
            nc.scalar.dma_start(out=sr[64:128, :], in_=src[0:64, :])
            ta = ropet.tile([128, TCH], BF16, tag="ropetmp", name=f"ra{n}")
            tb = ropet.tile([128, TCH], BF16, tag="ropetmp", name=f"rb{n}")
            nc.vector.tensor_mul(ta[:], src[:], csf_sb[:, sl])
            nc.vector.tensor_mul(tb[:], sr[:], snf_sb[:, sl])
            nc.vector.tensor_add(dst_full[:, sl], ta[:], tb[:])

        nc.gpsimd.collective_compute(
            "AllGather", mybir.AluOpType.bypass,
            replica_groups=[list(range(NCORES))],
            ins=[dmy_i[:].opt()], outs=[dmy_o[:].opt()])

        # ================= phase 1: qkvT = wqkv^T @ hsT =================
        # chunks descend: high-token pair (3,2) first so attention on chunk 3
        # can begin while the low pair computes
        pending_small = []
        for n in reversed(range(NCH)):
            pss = [ps1.tile([128, TCH], F32, tag="ps1t", name=f"ps1_{n}_{m}")
                   for m in range(4)]
            for k in range(KT):
                for m in range(4):
                    nc.tensor.matmul(pss[m][:],
                                     wqkv_sb[:, k, m * 128:(m + 1) * 128],
                                     hst_sb[:, k, n, :],
                                     start=(k == 0), stop=(k == KT - 1))
            if pending_small:
                pending_small.pop(0)()
            if n == 1:
                # attention constants: by now the hs low pair is streamed, so
                # these ride the sync ring without starving phase 1
                nc.sync.dma_start(out=kgt_sb[:], in_=KGT)
                nc.sync.dma_start(out=vg_sb[:],
                                  in_=VG.rearrange("(s p) d -> p s d", p=128))
                nc.sync.dma_start(out=ones_sb[:], in_=ONES)
                nc.sync.dma_start(out=onesr_sb[:], in_=ONESR)
                nc.sync.dma_start(
                    out=maskd_sb[:],
                    in_=MASKD2.rearrange("p (h c) -> p h c", h=2))
                nc.sync.dma_start(
                    out=maskl_sb[:],
                    in_=MASKL2.rearrange("p (h c) -> p h c", h=2))
            sl = bass.ds(n * TCH, TCH)
            q0c = chunkp.tile([128, TCH], BF16, tag="q0c")
            q1c = chunkp.tile([128, TCH], BF16, tag="q1c")
            kc = chunkp.tile([128, TCH], BF16, tag="kc")
            vc = chunkp.tile([128, TCH], BF16, tag="vc")
            nc.scalar.activation(q0c[:], pss[0][:], AF.Copy)
            nc.scalar.activation(q1c[:], pss[1][:], AF.Copy)
            nc.scalar.activation(kc[:], pss[2][:], AF.Copy)
            nc.vector.tensor_copy(vc[:], pss[3][:])

            rope_chunk(qrot[:, 0, :], q0c, n)
            rope_chunk(qrot[:, 1, :], q1c, n)
            rope_chunk(krot, kc, n)

            def small_ops(n=n, vc=vc, sl=sl, attn=False):
                # v transposes + gates for chunk n: emitted one chunk later so
                # the PE stream never waits on the DVE rope/copy latency.
                # attn=True -> running inside the attention phase (chunk 0's
                # deferred ops): draw PSUM from the attention score pool
                for j in range(4):
                    s = 4 * n + j
                    if attn:
                        pt = psA.tile([128, 128], BF16, tag="qka",
                                      name=f"pt{s}")
                    else:
                        pt = ps1.tile([128, 128], BF16, tag="ps1g",
                                      name=f"pt{s}", bufs=2)
                    nc.tensor.transpose(pt[:], vc[:, j * 128:(j + 1) * 128],
                                        idn_sb[:])
                    nc.vector.tensor_copy(vcur[:, s, :], pt[:])
                # gates for both heads in one PSUM tile: h0 -> partition 0,
                # h1 -> partition 32 (col-tiled), one Sigmoid serves both
                if attn:
                    gp = psA.tile([33, TCH], F32, tag="qka", name=f"gp{n}")
                else:
                    gp = ps1.tile([33, TCH], F32, tag="ps1s", name=f"gp{n}",
                                  bufs=1)
                nc.tensor.matmul(gp[0:1, :], wg_sb[:, 0:1], qrot[:, 0, sl],
                                 start=True, stop=True,
                                 tile_position=(0, 0))
                nc.tensor.matmul(gp[32:33, :], wg_sb[:, 1:2], qrot[:, 1, sl],
                                 start=True, stop=True,
                                 tile_position=(0, 32))
                if attn:
                    # mid-attention: a Sigmoid here would force two ACT
                    # table-set switches (~2.7us each, on every core).
                    # Compute it with the already-loaded Exp set instead:
                    # e = exp(-(z+b)); g = 1/(1+e); 1-g = e*g
                    ge = chunkp.tile([33, TCH], F32, tag="ge", name=f"ge{n}", bufs=1)
                    nc.scalar.activation(ge[:], gp[:], AF.Exp, scale=-1.0,
                                         bias=bgn_sb[:])
                    gt = chunkp.tile([33, TCH], F32, tag="gt", name=f"gt{n}", bufs=1)
                    nc.vector.tensor_scalar(gt[:], ge[:], 1.0, 1.0,
                                            mybir.AluOpType.mult,
                                            mybir.AluOpType.add)
                    gg = chunkp.tile([33, TCH], F32, tag="gg", name=f"gg{n}", bufs=1)
                    nc.vector.reciprocal_approx_fast(gg[:], gt[:])
                    nc.scalar.dma_start(out=gstack[0:33, n, :], in_=gg[:])
                    g1 = chunkp.tile([33, TCH], F32, tag="g1", name=f"g1{n}", bufs=1)
                    nc.vector.tensor_mul(g1[:], ge[:], gg[:])
                    nc.scalar.dma_start(out=gstack[64:97, n, :], in_=g1[:])
                else:
                    gst = chunkp.tile([33, TCH], F32, tag="gst",
                                      name=f"gst{n}")
                    nc.scalar.activation(gst[:], gp[:], AF.Sigmoid,
                                         bias=bg_sb[:])
                    nc.scalar.dma_start(out=gstack[0:33, n, :], in_=gst[:])
                    nc.scalar.dma_start(out=gstack[64:97, n, :], in_=gst[:])
                    nc.vector.tensor_scalar(gstack[64:97, n, :],
                                            gstack[64:97, n, :], -1.0, 1.0,
                                            mybir.AluOpType.mult,
                                            mybir.AluOpType.add)

            pending_small.append(small_ops)

        # chunk 0's small_ops are NOT flushed here: they run at the start of
        # the attention phase (drawing PSUM from the attention pools), so
        # attention QKs don't sit behind the phase-1 pool-release barrier.
        # chunkp/wqkvp stay alive through attention for vc(chunk0)/idn.
        ps1.release()
        hsp.release()

        # w_o prefetch: emitted now so the 8MB streams in during attention,
        # well before anything else joins the sync queue
        wop = tc.alloc_tile_pool(name="wop", bufs=4)
        wo_tiles = []
        wo_view = WO.rearrange("(k p) c -> p k c", p=128)
        for kq in range(4):
            wo_t = wop.tile([128, 4, HID], BF16, tag="wo", name=f"wo{kq}")
            nc.sync.dma_start(out=wo_t[:],
                              in_=wo_view[:, 4 * kq:4 * kq + 4, :])
            wo_tiles.append(wo_t)

        afull_hi = opool.tile([128, KT, TSL // 2], BF16)
        afull_lo = opool.tile([128, KT, TSL // 2], BF16)

        expp = tc.alloc_tile_pool(name="expp", bufs=4)
        accp = tc.alloc_tile_pool(name="accp", bufs=2)
        psA = tc.alloc_tile_pool(name="psA", bufs=2, space="PSUM")
        pspv = tc.alloc_tile_pool(name="pspv", bufs=1, space="PSUM")
        pvlp = tc.alloc_tile_pool(name="pvlp", bufs=1, space="PSUM")

        # chunk 0's deferred v-transposes + gates are flushed INSIDE the
        # first attention chunk (after its global pass) — see the loop

        # ============ phase 2: attention (global + local) ============
        # chunks descend so the high-token half finishes first and its
        # all-to-all overlaps the low-token half's compute.
        #
        # PSUM budget (8 banks): score pair tiles 2x2 + pv_g 2 + pv_l 2.
        # Both heads' QK scores of a step land in one fp32 pair tile and a
        # single Exp ACTIVATE covers both heads. Softmax sums never touch
        # the PE inside the step loop: the DVE accumulates the exp tiles
        # elementwise (off the critical path) and one ones-matmul per
        # pass/chunk reduces the accumulator across partitions into rows
        # 0/32 (global) and 64/96 (local) of a transient score-pool bank.
        pend_combine = [None]

        for n in reversed(range(NCH)):
            sl = bass.ds(n * TCH, TCH)
            pv_g = pspv.tile([128, 2, TCH], F32, tag="pv", name=f"pvg{n}")
            pv_l = pvlp.tile([128, 2, TCH], F32, tag="pvl", name=f"pvl{n}")
            exg_acc = accp.tile([128, 2, TCH], BF16, tag="acc",
                                name=f"exga{n}")
            exl_acc = accp.tile([128, 2, TCH], BF16, tag="accl",
                                name=f"exla{n}")
            smsb = combp.tile([128, TCH], F32, tag="smsb", name=f"smsb{n}")
            rcpt = combp.tile([128, TCH], F32, tag="rcpt", name=f"rcpt{n}")
            aglt = combp.tile([128, TCH], BF16, tag="aglt", name=f"aglt{n}")

            # ---- global pass over cached KV (both heads share k/v tiles);
            # diagonal tiles stream only the causally-live query extent
            ns = 4 * n + 4

            # software pipeline: emit step s's QK+exp, then step s-1's PV
            # (whose exp finished during this step's QKs) so the PE never
            # waits on the ACT engine inside a step
            def emit_pv_g(s, jo, ex):
                for h in range(2):
                    nc.tensor.matmul(pv_g[:, h, jo:], vg_sb[:, s, :],
                                     ex[:, h, jo:],
                                     start=(s == 0), stop=(s == ns - 1))

            hook_s = 2 if ns == 4 else 4
            prev_g = None
            for s in range(ns):
                if s == hook_s and pend_combine[0] is not None:
                    pend_combine[0]()
                    pend_combine[0] = None
                jo = max(0, (s - 4 * n) * 128)
                mv = bass.ds(n * TCH + jo, TCH - jo)
                qk = psA.tile([128, 2, TCH], F32, tag="qka",
                              name=f"qka_{n}_{s}")
                for h in range(2):
                    nc.tensor.matmul(qk[:, h, jo:],
                                     kgt_sb[:, s * 128:(s + 1) * 128],
                                     qrot[:, h, mv], start=True, stop=True)
                ex = expp.tile([128, 2, TCH], BF16, tag="ex",
                               name=f"exg_{n}_{s}")
                nc.scalar.activation(ex[:, :, jo:], qk[:, :, jo:],
                                     AF.Exp, scale=SCALE)
                if s >= 4 * n:
                    # multiplicative 0/1 causal mask on the in-block
                    # triangle, both heads at once
                    nc.vector.tensor_mul(ex[:, :, jo:jo + 128],
                                         ex[:, :, jo:jo + 128],
                                         maskd_sb[:])
                # softmax-sum partial: elementwise accumulate over key tiles
                # (s=0 always has jo=0, so the copy initializes fully).
                # Deferred one step so it never sits ahead of the next
                # step's mask multiply in the DVE FIFO.
                def acc_g(s=s, jo=jo, ex=ex):
                    if s == 0:
                        nc.vector.tensor_copy(exg_acc[:], ex[:])
                    else:
                        nc.vector.tensor_add(exg_acc[:, :, jo:],
                                             exg_acc[:, :, jo:],
                                             ex[:, :, jo:])
                if prev_g is not None:
                    emit_pv_g(*prev_g[:3])
                    prev_g[3]()
                prev_g = (s, jo, ex, acc_g)
            emit_pv_g(*prev_g[:3])
            prev_g[3]()

            # finalize global sums: one partition-reduce matmul per head into
            # rows 0/32 of a transient score-pool bank
            smg = psA.tile([128, 2, TCH], F32, tag="qka", name=f"smg{n}")
            for h in range(2):
                nc.tensor.matmul(smg[SMR[h]:SMR[h] + 1, 0, :], ones_sb[:],
                                 exg_acc[:, h, :], start=True, stop=True,
                                 tile_position=(0, SMR[h]))

            if pending_small:
                # chunk 0's deferred v-transposes + gates: by now their
                # inputs (vc/qrot of chunk 0) have long drained, and chunk
                # 3's global QKs didn't have to queue behind them
                pending_small.pop(0)(attn=True)

            if n == 0:
                # gather all-to-all #1 results now: the collective is done (or
                # nearly so), so this never head-blocks the sync DMA queue
                nc.sync.dma_start(
                    out=afull_hi[:, 0:4, :],
                    in_=a2ao_hi[0:2].rearrange("r (h p) c -> p (r h) c",
                                               h=2))
                for g2 in range(1, 4):
                    nc.sync.dma_start(
                        out=afull_hi[:, 4 * g2:4 * g2 + 4, :],
                        in_=a2ao_hi[2 * g2:2 * g2 + 2]
                        .rearrange("r (h p) c -> p (r h) c", h=2))

            # ---- global-combine DVE half: drain g-sum rows, reciprocal,
            # scale by gate; overlaps the local pass below
            nc.vector.tensor_copy(smsb[0:64, :], smg[0:64, 0, :])
            nc.vector.reciprocal_approx_fast(rcpt[0:64, :], smsb[0:64, :])
            nc.vector.tensor_mul(aglt[0:64, :], rcpt[0:64, :],
                                 gstack[0:64, n, :])

            # ---- local sliding-window pass over current KV: both heads of a
            # key tile share one fp32 bank ([2, 256] = 512 fp32)
            def emit_pv_l(t, e0, w, st, sp, exl):
                osl = bass.ds(e0, w)
                for h in range(2):
                    nc.tensor.matmul(pv_l[:, h, osl], vcur[:, t, :],
                                     exl[:, h, 0:w],
                                     start=st, stop=sp,
                                     skip_group_check=True)

            # one matmul per key tile over a padded 256-query extent with a
            # single shift-invariant band mask. Tiles 4n and 4n+2 are emitted
            # first with start=True: their extents exactly partition [0,512)
            # so the remaining tiles accumulate with start=False.
            # start=True only on the first tile: it row-clears the whole
            # bank, so tile 4n+2's disjoint region write-if-cleans correctly
            # and the overlapping odd tiles accumulate
            lt = [(4 * n, 0, 256, 0, True, False),
                  (4 * n + 2, 256, 256, 0, False, False),
                  (4 * n - 1, 0, 64, 128, False, True),
                  (4 * n + 1, 128, 192, 0, False, True),
                  (4 * n + 3, 384, 128, 0, False, True)]
            prev_l = None
            for (t, e0, w, m0, st, sp) in lt:
                if t < 0:
                    continue
                qsl = bass.ds(n * TCH + e0, w)
                qka_t = psA.tile([128, 2, TCH], F32, tag="qka",
                                 name=f"qkl_{n}_{t}")
                qkl = qka_t[:, 0, :].rearrange("p (h c) -> p h c", h=2)
                for h in range(2):
                    nc.tensor.matmul(qkl[:, h, 0:w],
                                     krot[:, t * 128:(t + 1) * 128],
                                     qrot[:, h, qsl],
                                     start=True, stop=True)
                exl = expp.tile([128, 2, 256], BF16, tag="exl",
                                name=f"exl_{n}_{t}")
                nc.scalar.activation(exl[:, :, 0:w], qkl[:, :, 0:w],
                                     AF.Exp, scale=SCALE)
                nc.vector.tensor_mul(exl[:, :, 0:w], exl[:, :, 0:w],
                                     maskl_sb[:, :, m0:m0 + w])
                # local softmax-sum partials: tiles 4n / 4n+2 initialize the
                # two disjoint halves; the overlapping tiles accumulate.
                # Deferred one tile (DVE FIFO: keep masks ahead of accs).
                def acc_l(t=t, e0=e0, w=w, exl=exl):
                    if t == 4 * n:
                        nc.vector.tensor_copy(exl_acc[:, :, 0:256],
                                              exl[:, :, 0:256])
                    elif t == 4 * n + 2:
                        nc.vector.tensor_copy(exl_acc[:, :, 256:512],
                                              exl[:, :, 0:256])
                    else:
                        nc.vector.tensor_add(exl_acc[:, :, e0:e0 + w],
                                             exl_acc[:, :, e0:e0 + w],
                                             exl[:, :, 0:w])
                if prev_l is not None:
                    emit_pv_l(*prev_l[:6])
                    prev_l[6]()
                prev_l = (t, e0, w, st, sp, exl, acc_l)
            emit_pv_l(*prev_l[:6])
            prev_l[6]()

            # finalize local sums into rows 64/96 and drain them early so the
            # transient bank frees quickly
            sml = psA.tile([128, 2, TCH], F32, tag="qka", name=f"sml{n}")
            for h in range(2):
                nc.tensor.matmul(sml[SMR[2 + h]:SMR[2 + h] + 1, 0, :],
                                 ones_sb[:], exl_acc[:, h, :],
                                 start=True, stop=True,
                                 tile_position=(0, SMR[2 + h]))
            nc.vector.tensor_copy(smsb[64:128, :], sml[64:128, 0, :])
            # full-partition op: custom-DVE reciprocal silently no-ops at
            # base partition 64; rows 0-63 recompute harmlessly
            nc.vector.reciprocal_approx_fast(rcpt[:], smsb[:])

            # ---- global-combine tail: broadcast per-token scales with PE
            # outer products and apply to pv_g (frees the pv_g banks)
            t1s = []
            for h in range(2):
                bcg = psA.tile([128, 2, TCH], F32, tag="qka",
                               name=f"bcg{h}_{n}")[:, 0, :]
                r0 = SMR[h]
                nc.tensor.matmul(bcg[:], onesr_sb[r0:r0 + 1, :],
                                 aglt[r0:r0 + 1, :],
                                 start=True, stop=True,
                                 tile_position=(r0, 0))
                bcgs = combp.tile([128, TCH], BF16, tag="bcs",
                                  name=f"bcgs{h}_{n}", bufs=4)
                nc.vector.tensor_copy(bcgs[:], bcg[:])
                t1 = combp.tile([128, TCH], F32, tag="comb",
                                name=f"t1_{h}_{n}", bufs=4)
                nc.vector.tensor_mul(t1[:], pv_g[:, h, :], bcgs[:])
                t1s.append(t1)
            # full-partition gate scale (rows 64:128 feed the bcl broadcasts
            # in the deferred combine; rows 0:64 recompute identical values)
            nc.vector.tensor_mul(aglt[:], rcpt[:], gstack[:, n, :])

            # ---- local-combine: deferred into the next chunk's global pass
            # so the scale/broadcast chain hides behind fresh PE work
            def local_combine(n=n, pv_l=pv_l, aglt=aglt, t1s=t1s):
                for h in range(2):
                    bcl = psA.tile([128, 2, TCH], F32, tag="qka",
                                   name=f"bcl{h}_{n}")[:, 0, :]
                    r0 = SMR[2 + h]
                    nc.tensor.matmul(bcl[:], onesr_sb[r0:r0 + 1, :],
                                     aglt[r0:r0 + 1, :],
                                     start=True, stop=True,
                                     tile_position=(r0, 0))
                    bcls = combp.tile([128, TCH], BF16, tag="bcs",
                                      name=f"bcls{h}_{n}", bufs=4)
                    nc.vector.tensor_copy(bcls[:], bcl[:])
                    t2 = combp.tile([128, TCH], F32, tag="comb",
                                    name=f"t2_{h}_{n}", bufs=4)
                    ao = aoutp.tile([128, TCH], BF16, tag="aout",
                                    name=f"ao{2 * n + h}")
                    nc.vector.tensor_mul(t2[:], pv_l[:, h, :], bcls[:])
                    nc.vector.tensor_add(ao[:], t1s[h][:], t2[:])
                    # ship the finished [128, 512] block to a2a staging with
                    # ONE strided SWDGE DMA (gpsimd queue): keeps the sync
                    # HWDGE ring free and orders ahead of the trigger below.
                    # token 1024+128c (hi) / 128c (lo) lives in chunk n at
                    # column offset 128j
                    buf = a2ai_hi if n >= 2 else a2ai_lo
                    c0 = (n - 2) * 4 if n >= 2 else n * 4
                    nc.gpsimd.dma_start(
                        out=buf[c0:c0 + 4, h * D:(h + 1) * D, :]
                        .rearrange("j p c -> p j c"),
                        in_=ao[:].rearrange("p (j c) -> p j c", j=4))
                if n == 2:
                    # all-to-all #1: high-token halves (overlaps chunks 1,0)
                    nc.gpsimd.collective_compute(
                        "AllToAll", mybir.AluOpType.bypass,
                        replica_groups=[list(range(NCORES))],
                        ins=[a2ai_hi[:].opt()], outs=[a2ao_hi[:].opt()])

            pend_combine[0] = local_combine

        pend_combine[0]()
        pend_combine[0] = None

        pvlp.release()
        pspv.release()
        psA.release()
        accp.release()
        expp.release()

        # ========= phase 3: all-to-all #2 (low-token halves) =========
        nc.gpsimd.collective_compute(
            "AllToAll", mybir.AluOpType.bypass,
            replica_groups=[list(range(NCORES))],
            ins=[a2ai_lo[:].opt()], outs=[a2ao_lo[:].opt()])
        nc.sync.dma_start(
            out=afull_lo[:, 0:4, :],
            in_=a2ao_lo[0:2].rearrange("r (h p) c -> p (r h) c", h=2))
        for g2 in range(1, 4):
            nc.sync.dma_start(
                out=afull_lo[:, 4 * g2:4 * g2 + 4, :],
                in_=a2ao_lo[2 * g2:2 * g2 + 2]
                .rearrange("r (h p) c -> p (r h) c", h=2))

        pso = tc.alloc_tile_pool(name="pso", bufs=8, space="PSUM")

        # ============ phase 4: o_proj for our token slice ============
        # OUT rows 0-127 = low half-slice, rows 128-255 = high half-slice.
        # hi half first: it only needs all-to-all #1, so the PE works while
        # all-to-all #2 is still in flight. e-outer / k-inner: each 512-col
        # slice drains right after its 16 accumulates so the PSUM copy and
        # OUT store (scalar HWDGE ring; ACT is idle here) overlap the rest.
        for tt, afull in ((1, afull_hi), (0, afull_lo)):
            for ep in range(0, NCH, 2):
                pos = [pso.tile([128, TCH], F32, tag="po",
                                name=f"po_{tt}_{ep + i}") for i in range(2)]
                for k in range(KT):
                    # both e-slices of the pair share one LDWEIGHTS of
                    # afull[:, k, :]
                    for i in range(2):
                        e = ep + i
                        nc.tensor.matmul(
                            pos[i][:],
                            afull[:, k, :],
                            wo_tiles[k // 4][:, k % 4,
                                             e * TCH:(e + 1) * TCH],
                            start=(k == 0), stop=(k == KT - 1))
                for i in range(2):
                    e = ep + i
                    ot = osb.tile([128, TCH], BF16, tag="ot",
                                  name=f"ot{tt}_{e}")
                    nc.vector.tensor_copy(ot[:], pos[i][:])
                    nc.scalar.dma_start(
                        out=OUT[tt * 128:(tt + 1) * 128,
                                e * TCH:(e + 1) * TCH],
                        in_=ot[:])
        pso.release()
        wop.release()
        chunkp.release()
        wqkvp.release()
        combp.release()
        ropet.release()
        work.release()
        osb.release()
        opool.release()
        aoutp.release()
        dram.release()
        const.release()

    nc.compile()
    return nc


def _host_prep(hidden_states, positions, k_global, v_global, w_qkv, w_o,
               w_gate, b_gate):
    """Layout-only host transforms + constant tables -> per-core in_maps."""
    f32 = np.float32
    bf16 = ml_dtypes.bfloat16
    hs = np.asarray(hidden_states, f32)
    pos = np.asarray(positions)
    kg = np.asarray(k_global, f32)
    vg = np.asarray(v_global, f32)
    wqkv = np.asarray(w_qkv, f32)
    wo = np.ascontiguousarray(np.asarray(w_o, f32).astype(bf16))
    wg = np.asarray(w_gate, f32)
    bg = np.asarray(b_gate, f32)

    hst = np.ascontiguousarray(hs.T.astype(bf16))

    half = D // 2
    inv_freq = (THETA ** (-np.arange(half, dtype=f32) / half)).astype(f32)
    ang = pos.astype(f32)[:, None] * inv_freq[None, :]
    cos_t = np.cos(ang).astype(f32).T       # [64, T]
    sin_t = np.sin(ang).astype(f32).T
    csf = np.ascontiguousarray(np.concatenate([cos_t, cos_t], axis=0)).astype(bf16)
    snf = np.ascontiguousarray(np.concatenate([-sin_t, sin_t], axis=0)).astype(bf16)

    p = np.arange(128, dtype=np.int64)[:, None]   # key row within tile
    q = np.arange(128, dtype=np.int64)[None, :]   # query col within block
    # within-block causal triangle for global diagonal tiles (0/1, applied
    # multiplicatively to the exp'd scores); duplicated for the two heads
    maskd = np.where(q >= p, 1.0, 0.0).astype(bf16)
    maskd2 = np.ascontiguousarray(np.concatenate([maskd, maskd], axis=1))
    # canonical local band mask: key row k vs query offset e within a
    # 256-query extent starting at the key tile's base; head-duplicated
    e = np.arange(256, dtype=np.int64)[None, :]
    maskl = np.where((e - p >= 0) & (e - p <= WIN), 1.0, 0.0).astype(bf16)
    maskl2 = np.ascontiguousarray(np.concatenate([maskl, maskl], axis=1))

    ones = np.ones((128, 1), bf16)
    onesr = np.ones((128, 128), bf16)
    idn = np.eye(128, dtype=bf16)

    in_maps = []
    for c in range(NCORES):
        g = c // 2
        wq = wqkv[:, 2 * c * D:(2 * c + 2) * D]
        wk = wqkv[:, HQ * D + g * D:HQ * D + (g + 1) * D]
        wv = wqkv[:, (HQ + HK) * D + g * D:(HQ + HK) * D + (g + 1) * D]
        bgv = np.zeros((33, 1), f32)
        bgv[0, 0] = bg[2 * c]
        bgv[32, 0] = bg[2 * c + 1]
        in_maps.append({
            "HST": hst,
            "WQKV": np.ascontiguousarray(
                np.concatenate([wq, wk, wv], axis=1).astype(bf16)),
            "KGT": np.ascontiguousarray(kg[:, g * D:(g + 1) * D].T.astype(bf16)),
            "VG": np.ascontiguousarray(vg[:, g * D:(g + 1) * D].astype(bf16)),
            "WO": wo,
            "WG": np.ascontiguousarray(wg[:, 2 * c:2 * c + 2].astype(bf16)),
            "BG": bgv,
            "CSF": csf,
            "SNF": snf,
            "ONES": ones,
            "ONESR": onesr,
            "IDN": idn,
            "MASKD2": maskd2,
            "MASKL2": maskl2,
        })
    return in_maps


def kernel(**inputs):
    if "nc" not in _CACHE:
        _CACHE["nc"] = _build()
    nc = _CACHE["nc"]
    in_maps = _host_prep(**inputs)
    res = run_bass_kernel_spmd(nc, in_maps, core_ids=list(range(NCORES)))
    out = np.empty((T, HID), np.float32)
    for c in range(NCORES):
        o = np.asarray(res.results[c]["OUT"]).astype(np.float32)
        out[128 * c:128 * (c + 1)] = o[0:128]
        out[1024 + 128 * c:1024 + 128 * (c + 1)] = o[128:256]
    return out


# revision 41
# speedup vs baseline: 1.0550x; 1.0550x over previous
"""Trainium2 Bass kernel for LoopCoderAttention (sparse_attention).

Head-sharded tensor parallelism over 8 NeuronCores:
  core c owns query heads {2c, 2c+1} and KV head c//2.
All on-device tensors live in transposed [feature, token] layout so every
matmul contracts along the partition dim with zero on-device transposes
(except v, which needs one PE transpose per 128-tile).

v4 notes (on top of the v3 streaming/queue layout):
 - attention scores for BOTH heads of a step land in adjacent PSUM banks
   of one persistent 4-bank "ring" tile, so a single Exp ACTIVATE covers
   both heads (the ACT engine was the steady-state bottleneck at 2 calls
   per step). Global steps ping-pong bank pairs (0,1)/(2,3); the local
   pass packs both heads into ONE bank per key tile (256+256 columns)
   and accumulates pv_l into ring banks 2,3 — freeing enough PSUM for
   the wider score tiles (ring 4 + pv_g 2 + sums 1 + bcast 1 = 8 banks).
 - causal/band masks are applied with one DVE multiply per step against
   head-duplicated mask tables (MASKD2/MASKL2).
 - per-token scale broadcasts (ones outer products) serialize through a
   dedicated 1-bank "bcast" tile instead of stealing score banks.
 - a2a staging is 2 strided SWDGE DMAs per chunk-combine (gpsimd queue),
   immediately followed in the same FIFO by the collective trigger.

o_proj: a 2MB AllToAll reshards attention output from head-sharded to
token-sharded; each core then runs the full 2048-deep contraction for its
256-token slice (the "all-reduce" happens inside the matmul accumulation).
"""
import sys
sys.path.insert(0, '/opt/trn_rl_repo')
import numpy as np
import ml_dtypes
import concourse.bass as bass
import concourse.mybir as mybir
import concourse.tile as tile
from concourse import bacc
from concourse.bass_utils import run_bass_kernel_spmd

T = 2048
HID = 2048
HQ = 16
HK = 4
D = 128
WIN = 64
THETA = 10000.0
SCALE = D ** -0.5
NCORES = 8
TCH = 512                 # t-chunk (matmul free dim)
NCH = T // TCH            # 4 chunks
KT = HID // 128           # 16 k-tiles for 2048-deep contractions
ST = T // 128             # 16 s-tiles
TSL = T // NCORES         # 256-token output slice per core
MASKV = -1e9

F32 = mybir.dt.float32
BF16 = mybir.dt.bfloat16
AF = mybir.ActivationFunctionType

_CACHE = {}


def _build():
    nc = bacc.Bacc("TRN2", target_bir_lowering=False, debug=False,
                   num_devices=NCORES)
    HST = nc.dram_tensor("HST", [HID, T], BF16, kind="ExternalInput").ap()
    WQKV = nc.dram_tensor("WQKV", [HID, 512], BF16, kind="ExternalInput").ap()
    KGT = nc.dram_tensor("KGT", [D, T], BF16, kind="ExternalInput").ap()
    VG = nc.dram_tensor("VG", [T, D], BF16, kind="ExternalInput").ap()
    WO = nc.dram_tensor("WO", [HID, HID], BF16, kind="ExternalInput").ap()
    WG = nc.dram_tensor("WG", [D, 2], BF16, kind="ExternalInput").ap()
    BG = nc.dram_tensor("BG", [33, 1], F32, kind="ExternalInput").ap()
    CSF = nc.dram_tensor("CSF", [128, T], BF16, kind="ExternalInput").ap()
    SNF = nc.dram_tensor("SNF", [128, T], BF16, kind="ExternalInput").ap()
    ONES = nc.dram_tensor("ONES", [128, 1], BF16, kind="ExternalInput").ap()
    ONESR = nc.dram_tensor("ONESR", [128, 128], BF16, kind="ExternalInput").ap()
    IDN = nc.dram_tensor("IDN", [128, 128], BF16, kind="ExternalInput").ap()
    MASKD2 = nc.dram_tensor("MASKD2", [128, 256], BF16,
                            kind="ExternalInput").ap()
    MASKL2 = nc.dram_tensor("MASKL2", [128, 512], BF16,
                            kind="ExternalInput").ap()
    OUT = nc.dram_tensor("OUT", [TSL, HID], BF16, kind="ExternalOutput").ap()

    with tile.TileContext(nc) as tc:
        # pools are a strict stack: creation order is the reverse of the
        # release order at each phase boundary
        const = tc.alloc_tile_pool(name="const", bufs=1)
        dram = tc.alloc_tile_pool(name="dram", bufs=1, space="DRAM")
        aoutp = tc.alloc_tile_pool(name="aoutp", bufs=3)
        opool = tc.alloc_tile_pool(name="opool", bufs=1)
        osb = tc.alloc_tile_pool(name="osb", bufs=3)
        work = tc.alloc_tile_pool(name="work", bufs=1)
        ropet = tc.alloc_tile_pool(name="ropet", bufs=2)
        combp = tc.alloc_tile_pool(name="combp", bufs=2)
        wqkvp = tc.alloc_tile_pool(name="wqkvp", bufs=1)
        chunkp = tc.alloc_tile_pool(name="chunkp", bufs=2)
        hsp = tc.alloc_tile_pool(name="hsp", bufs=1)
        ps1 = tc.alloc_tile_pool(name="ps1", bufs=5, space="PSUM")

        # ---- phase-1 input streaming, interleaved across both HWDGE rings
        # so the first matmul can start right after the ~7us NRT preamble:
        #   scalar ring: wqkv, csf/snf halves, small consts
        #   sync ring:   hs high-token pair (small slabs first), low pair,
        #                then attention constants mid-loop
        wqkv_sb = wqkvp.tile([128, KT, 512], BF16)
        wqkv_view = WQKV.rearrange("(k p) c -> p k c", p=128)
        hst_sb = hsp.tile([128, KT, 4, TCH], BF16)
        hst_view = HST.rearrange("(k p) t -> p k t", p=128)
        nc.scalar.dma_start(out=wqkv_sb[:, 0:2, :], in_=wqkv_view[:, 0:2, :])
        nc.sync.dma_start(out=hst_sb[:, 0:1, 2:4, :],
                          in_=hst_view[:, 0:1, 1024:2048])
        nc.sync.dma_start(out=hst_sb[:, 1:2, 2:4, :],
                          in_=hst_view[:, 1:2, 1024:2048])
        nc.scalar.dma_start(out=wqkv_sb[:, 2:8, :], in_=wqkv_view[:, 2:8, :])
        for kg in range(1, 8):
            nc.sync.dma_start(out=hst_sb[:, 2 * kg:2 * kg + 2, 2:4, :],
                              in_=hst_view[:, 2 * kg:2 * kg + 2, 1024:2048])
        nc.scalar.dma_start(out=wqkv_sb[:, 8:16, :], in_=wqkv_view[:, 8:16, :])
        csf_sb = wqkvp.tile([128, T], BF16)
        snf_sb = wqkvp.tile([128, T], BF16)
        nc.scalar.dma_start(out=csf_sb[:, 1024:2048], in_=CSF[:, 1024:2048])
        nc.scalar.dma_start(out=snf_sb[:, 1024:2048], in_=SNF[:, 1024:2048])
        idn_sb = wqkvp.tile([128, 128], BF16)
        nc.scalar.dma_start(out=idn_sb[:], in_=IDN)
        wg_sb = const.tile([D, 2], BF16)
        nc.scalar.dma_start(out=wg_sb[:], in_=WG)
        bg_sb = const.tile([33, 1], F32)
        nc.scalar.dma_start(out=bg_sb[:], in_=BG)
        # low-token hs pair + low cos/sin: streamed while chunks 3/2 compute
        for kg in range(4):
            nc.sync.dma_start(out=hst_sb[:, 4 * kg:4 * kg + 4, 0:2, :],
                              in_=hst_view[:, 4 * kg:4 * kg + 4, 0:1024])
        nc.scalar.dma_start(out=csf_sb[:, 0:1024], in_=CSF[:, 0:1024])
        nc.scalar.dma_start(out=snf_sb[:, 0:1024], in_=SNF[:, 0:1024])
        # attention-phase constants (emitted mid phase-1 loop, see below)
        kgt_sb = const.tile([D, T], BF16)
        vg_sb = const.tile([128, ST, D], BF16)
        ones_sb = const.tile([128, 1], BF16)
        onesr_sb = const.tile([128, 128], BF16)
        maskd_sb = const.tile([128, 2, 128], BF16)
        maskl_sb = const.tile([128, 2, 256], BF16)

        # ---- persistent work tiles (through attention) ----
        qrot = work.tile([128, 2, T], BF16)
        krot = work.tile([128, T], BF16)
        vcur = work.tile([128, ST, D], BF16)   # current v in [s, d] tiles
        # gates staged at the partitions where the softmax-sum rows land:
        # row 0 = g_h0, 32 = g_h1, 64 = 1-g_h0, 96 = 1-g_h1 (per chunk n)
        gstack = work.tile([128, NCH, TCH], F32)
        SMR = (0, 32, 64, 96)

        # tiny dummy collective fired at kernel start: absorbs the cc-stream
        # init barrier and the ~11.5us first-trigger penalty under phase 1,
        # so the real all-to-alls start promptly
        dmy_i = dram.tile([NCORES, 16], BF16)
        dmy_o = dram.tile([NCORES, NCORES, 16], BF16)
        a2ai_hi = dram.tile([NCORES, 2 * D, TSL // 2], BF16)
        a2ao_hi = dram.tile([NCORES, 2 * D, TSL // 2], BF16)
        a2ai_lo = dram.tile([NCORES, 2 * D, TSL // 2], BF16)
        a2ao_lo = dram.tile([NCORES, 2 * D, TSL // 2], BF16)

        def rope_chunk(dst_full, src, n):
            """dst_full[:, n*TCH:...] = neox-rope of chunk tile src [128, TCH].

            rot = src * [cos;cos] + rot90(src) * [-sin;sin], where rot90 swaps
            the two 64-partition halves (built with two SBUF->SBUF DMAs since
            DVE ops require matching base partitions).
            """
            sl = bass.ds(n * TCH, TCH)
            sr = ropet.tile([128, TCH], BF16, tag="ropesr", name=f"sr{n}")
            # scalar HWDGE ring: chains right behind the ACT copy that
            # produced src, and keeps the sync ring free for bulk loads
            nc.scalar.dma_start(out=sr[0:64, :], in_=src[64:128, :])
            nc.scalar.dma_start(out=sr[64:128, :], in_=src[0:64, :])
            ta = ropet.tile([128, TCH], BF16, tag="ropetmp", name=f"ra{n}")
            tb = ropet.tile([128, TCH], BF16, tag="ropetmp", name=f"rb{n}")
            nc.vector.tensor_mul(ta[:], src[:], csf_sb[:, sl])
            nc.vector.tensor_mul(tb[:], sr[:], snf_sb[:, sl])
            nc.vector.tensor_add(dst_full[:, sl], ta[:], tb[:])

        nc.gpsimd.collective_compute(
            "AllGather", mybir.AluOpType.bypass,
            replica_groups=[list(range(NCORES))],
            ins=[dmy_i[:].opt()], outs=[dmy_o[:].opt()])

        # ================= phase 1: qkvT = wqkv^T @ hsT =================
        # chunks descend: high-token pair (3,2) first so attention on chunk 3
        # can begin while the low pair computes
        pending_small = []
        for n in reversed(range(NCH)):
            pss = [ps1.tile([128, TCH], F32, tag="ps1t", name=f"ps1_{n}_{m}")
                   for m in range(4)]
            for k in range(KT):
                for m in range(4):
                    nc.tensor.matmul(pss[m][:],
                                     wqkv_sb[:, k, m * 128:(m + 1) * 128],
                                     hst_sb[:, k, n, :],
                                     start=(k == 0), stop=(k == KT - 1))
            if pending_small:
                pending_small.pop(0)()
            if n == 1:
                # attention constants: by now the hs low pair is streamed, so
                # these ride the sync ring without starving phase 1
                nc.sync.dma_start(out=kgt_sb[:], in_=KGT)
                nc.sync.dma_start(out=vg_sb[:],
                                  in_=VG.rearrange("(s p) d -> p s d", p=128))
                nc.sync.dma_start(out=ones_sb[:], in_=ONES)
                nc.sync.dma_start(out=onesr_sb[:], in_=ONESR)
                nc.sync.dma_start(
                    out=maskd_sb[:],
                    in_=MASKD2.rearrange("p (h c) -> p h c", h=2))
                nc.sync.dma_start(
                    out=maskl_sb[:],
                    in_=MASKL2.rearrange("p (h c) -> p h c", h=2))
            sl = bass.ds(n * TCH, TCH)
            q0c = chunkp.tile([128, TCH], BF16, tag="q0c")
            q1c = chunkp.tile([128, TCH], BF16, tag="q1c")
            kc = chunkp.tile([128, TCH], BF16, tag="kc")
            vc = chunkp.tile([128, TCH], BF16, tag="vc")
            nc.scalar.activation(q0c[:], pss[0][:], AF.Copy)
            nc.scalar.activation(q1c[:], pss[1][:], AF.Copy)
            nc.scalar.activation(kc[:], pss[2][:], AF.Copy)
            nc.vector.tensor_copy(vc[:], pss[3][:])

            rope_chunk(qrot[:, 0, :], q0c, n)
            rope_chunk(qrot[:, 1, :], q1c, n)
            rope_chunk(krot, kc, n)

            def small_ops(n=n, vc=vc, sl=sl, attn=False):
                # v transposes + gates for chunk n: emitted one chunk later so
                # the PE stream never waits on the DVE rope/copy latency.
                # attn=True -> running inside the attention phase (chunk 0's
                # deferred ops): draw PSUM from the attention score pool
                for j in range(4):
                    s = 4 * n + j
                    if attn:
                        pt = psA.tile([128, 128], BF16, tag="qka",
                                      name=f"pt{s}")
                    else:
                        pt = ps1.tile([128, 128], BF16, tag="ps1g",
                                      name=f"pt{s}", bufs=2)
                    nc.tensor.transpose(pt[:], vc[:, j * 128:(j + 1) * 128],
                                        idn_sb[:])
                    nc.vector.tensor_copy(vcur[:, s, :], pt[:])
                # gates for both heads in one PSUM tile: h0 -> partition 0,
                # h1 -> partition 32 (col-tiled), one Sigmoid serves both
                if attn:
                    gp = psA.tile([33, TCH], F32, tag="qka", name=f"gp{n}")
                else:
                    gp = ps1.tile([33, TCH], F32, tag="ps1s", name=f"gp{n}",
                                  bufs=1)
                nc.tensor.matmul(gp[0:1, :], wg_sb[:, 0:1], qrot[:, 0, sl],
                                 start=True, stop=True,
                                 tile_position=(0, 0))
                nc.tensor.matmul(gp[32:33, :], wg_sb[:, 1:2], qrot[:, 1, sl],
                                 start=True, stop=True,
                                 tile_position=(0, 32))
                gst = chunkp.tile([33, TCH], F32, tag="gst", name=f"gst{n}")
                nc.scalar.activation(gst[:], gp[:], AF.Sigmoid,
                                     bias=bg_sb[:])
                nc.scalar.dma_start(out=gstack[0:33, n, :], in_=gst[:])
                nc.scalar.dma_start(out=gstack[64:97, n, :], in_=gst[:])
                nc.vector.tensor_scalar(gstack[64:97, n, :],
                                        gstack[64:97, n, :], -1.0, 1.0,
                                        mybir.AluOpType.mult,
                                        mybir.AluOpType.add)

            pending_small.append(small_ops)

        # chunk 0's small_ops are NOT flushed here: they run at the start of
        # the attention phase (drawing PSUM from the attention pools), so
        # attention QKs don't sit behind the phase-1 pool-release barrier.
        # chunkp/wqkvp stay alive through attention for vc(chunk0)/idn.
        ps1.release()
        hsp.release()

        # w_o prefetch: emitted now so the 8MB streams in during attention,
        # well before anything else joins the sync queue
        wop = tc.alloc_tile_pool(name="wop", bufs=4)
        wo_tiles = []
        wo_view = WO.rearrange("(k p) c -> p k c", p=128)
        for kq in range(4):
            wo_t = wop.tile([128, 4, HID], BF16, tag="wo", name=f"wo{kq}")
            nc.sync.dma_start(out=wo_t[:],
                              in_=wo_view[:, 4 * kq:4 * kq + 4, :])
            wo_tiles.append(wo_t)

        afull_hi = opool.tile([128, KT, TSL // 2], BF16)
        afull_lo = opool.tile([128, KT, TSL // 2], BF16)

        expp = tc.alloc_tile_pool(name="expp", bufs=4)
        accp = tc.alloc_tile_pool(name="accp", bufs=2)
        psA = tc.alloc_tile_pool(name="psA", bufs=2, space="PSUM")
        pspv = tc.alloc_tile_pool(name="pspv", bufs=1, space="PSUM")
        pvlp = tc.alloc_tile_pool(name="pvlp", bufs=1, space="PSUM")

        # chunk 0's deferred v-transposes + gates run here, overlapping the
        # first attention chunk instead of serializing behind the phase-1
        # pool barrier
        pending_small[0](attn=True)
        pending_small.clear()

        # ============ phase 2: attention (global + local) ============
        # chunks descend so the high-token half finishes first and its
        # all-to-all overlaps the low-token half's compute.
        #
        # PSUM budget (8 banks): score pair tiles 2x2 + pv_g 2 + pv_l 2.
        # Both heads' QK scores of a step land in one fp32 pair tile and a
        # single Exp ACTIVATE covers both heads. Softmax sums never touch
        # the PE inside the step loop: the DVE accumulates the exp tiles
        # elementwise (off the critical path) and one ones-matmul per
        # pass/chunk reduces the accumulator across partitions into rows
        # 0/32 (global) and 64/96 (local) of a transient score-pool bank.
        pend_combine = [None]

        for n in reversed(range(NCH)):
            sl = bass.ds(n * TCH, TCH)
            pv_g = pspv.tile([128, 2, TCH], F32, tag="pv", name=f"pvg{n}")
            pv_l = pvlp.tile([128, 2, TCH], F32, tag="pvl", name=f"pvl{n}")
            exg_acc = accp.tile([128, 2, TCH], BF16, tag="acc",
                                name=f"exga{n}")
            exl_acc = accp.tile([128, 2, TCH], BF16, tag="accl",
                                name=f"exla{n}")
            smsb = combp.tile([128, TCH], F32, tag="smsb", name=f"smsb{n}")
            rcpt = combp.tile([128, TCH], F32, tag="rcpt", name=f"rcpt{n}")
            aglt = combp.tile([128, TCH], BF16, tag="aglt", name=f"aglt{n}")

            # ---- global pass over cached KV (both heads share k/v tiles);
            # diagonal tiles stream only the causally-live query extent
            ns = 4 * n + 4

            # software pipeline: emit step s's QK+exp, then step s-1's PV
            # (whose exp finished during this step's QKs) so the PE never
            # waits on the ACT engine inside a step
            def emit_pv_g(s, jo, ex):
                for h in range(2):
                    nc.tensor.matmul(pv_g[:, h, jo:], vg_sb[:, s, :],
                                     ex[:, h, jo:],
                                     start=(s == 0), stop=(s == ns - 1))

            hook_s = 2 if ns == 4 else 4
            prev_g = None
            for s in range(ns):
                if s == hook_s and pend_combine[0] is not None:
                    pend_combine[0]()
                    pend_combine[0] = None
                jo = max(0, (s - 4 * n) * 128)
                mv = bass.ds(n * TCH + jo, TCH - jo)
                qk = psA.tile([128, 2, TCH], F32, tag="qka",
                              name=f"qka_{n}_{s}")
                for h in range(2):
                    nc.tensor.matmul(qk[:, h, jo:],
                                     kgt_sb[:, s * 128:(s + 1) * 128],
                                     qrot[:, h, mv], start=True, stop=True)
                ex = expp.tile([128, 2, TCH], BF16, tag="ex",
                               name=f"exg_{n}_{s}")
                nc.scalar.activation(ex[:, :, jo:], qk[:, :, jo:],
                                     AF.Exp, scale=SCALE)
                if s >= 4 * n:
                    # multiplicative 0/1 causal mask on the in-block
                    # triangle, both heads at once
                    nc.vector.tensor_mul(ex[:, :, jo:jo + 128],
                                         ex[:, :, jo:jo + 128],
                                         maskd_sb[:])
                # softmax-sum partial: elementwise accumulate over key tiles
                # (s=0 always has jo=0, so the copy initializes fully).
                # Deferred one step so it never sits ahead of the next
                # step's mask multiply in the DVE FIFO.
                def acc_g(s=s, jo=jo, ex=ex):
                    if s == 0:
                        nc.vector.tensor_copy(exg_acc[:], ex[:])
                    else:
                        nc.vector.tensor_add(exg_acc[:, :, jo:],
                                             exg_acc[:, :, jo:],
                                             ex[:, :, jo:])
                if prev_g is not None:
                    emit_pv_g(*prev_g[:3])
                    prev_g[3]()
                prev_g = (s, jo, ex, acc_g)
            emit_pv_g(*prev_g[:3])
            prev_g[3]()

            # finalize global sums: one partition-reduce matmul per head into
            # rows 0/32 of a transient score-pool bank
            smg = psA.tile([128, 2, TCH], F32, tag="qka", name=f"smg{n}")
            for h in range(2):
                nc.tensor.matmul(smg[SMR[h]:SMR[h] + 1, 0, :], ones_sb[:],
                                 exg_acc[:, h, :], start=True, stop=True,
                                 tile_position=(0, SMR[h]))

            if n == 0:
                # gather all-to-all #1 results now: the collective is done (or
                # nearly so), so this never head-blocks the sync DMA queue
                nc.sync.dma_start(
                    out=afull_hi[:, 0:4, :],
                    in_=a2ao_hi[0:2].rearrange("r (h p) c -> p (r h) c",
                                               h=2))
                for g2 in range(1, 4):
                    nc.sync.dma_start(
                        out=afull_hi[:, 4 * g2:4 * g2 + 4, :],
                        in_=a2ao_hi[2 * g2:2 * g2 + 2]
                        .rearrange("r (h p) c -> p (r h) c", h=2))

            # ---- global-combine DVE half: drain g-sum rows, reciprocal,
            # scale by gate; overlaps the local pass below
            nc.vector.tensor_copy(smsb[0:64, :], smg[0:64, 0, :])
            nc.vector.reciprocal_approx_fast(rcpt[0:64, :], smsb[0:64, :])
            nc.vector.tensor_mul(aglt[0:64, :], rcpt[0:64, :],
                                 gstack[0:64, n, :])

            # ---- local sliding-window pass over current KV: both heads of a
            # key tile share one fp32 bank ([2, 256] = 512 fp32)
            def emit_pv_l(t, e0, w, st, sp, exl):
                osl = bass.ds(e0, w)
                for h in range(2):
                    nc.tensor.matmul(pv_l[:, h, osl], vcur[:, t, :],
                                     exl[:, h, 0:w],
                                     start=st, stop=sp,
                                     skip_group_check=True)

            # one matmul per key tile over a padded 256-query extent with a
            # single shift-invariant band mask. Tiles 4n and 4n+2 are emitted
            # first with start=True: their extents exactly partition [0,512)
            # so the remaining tiles accumulate with start=False.
            # start=True only on the first tile: it row-clears the whole
            # bank, so tile 4n+2's disjoint region write-if-cleans correctly
            # and the overlapping odd tiles accumulate
            lt = [(4 * n, 0, 256, 0, True, False),
                  (4 * n + 2, 256, 256, 0, False, False),
                  (4 * n - 1, 0, 64, 128, False, True),
                  (4 * n + 1, 128, 192, 0, False, True),
                  (4 * n + 3, 384, 128, 0, False, True)]
            prev_l = None
            for (t, e0, w, m0, st, sp) in lt:
                if t < 0:
                    continue
                qsl = bass.ds(n * TCH + e0, w)
                qka_t = psA.tile([128, 2, TCH], F32, tag="qka",
                                 name=f"qkl_{n}_{t}")
                qkl = qka_t[:, 0, :].rearrange("p (h c) -> p h c", h=2)
                for h in range(2):
                    nc.tensor.matmul(qkl[:, h, 0:w],
                                     krot[:, t * 128:(t + 1) * 128],
                                     qrot[:, h, qsl],
                                     start=True, stop=True)
                exl = expp.tile([128, 2, 256], BF16, tag="exl",
                                name=f"exl_{n}_{t}")
                nc.scalar.activation(exl[:, :, 0:w], qkl[:, :, 0:w],
                                     AF.Exp, scale=SCALE)
                nc.vector.tensor_mul(exl[:, :, 0:w], exl[:, :, 0:w],
                                     maskl_sb[:, :, m0:m0 + w])
                # local softmax-sum partials: tiles 4n / 4n+2 initialize the
                # two disjoint halves; the overlapping tiles accumulate.
                # Deferred one tile (DVE FIFO: keep masks ahead of accs).
                def acc_l(t=t, e0=e0, w=w, exl=exl):
                    if t == 4 * n:
                        nc.vector.tensor_copy(exl_acc[:, :, 0:256],
                                              exl[:, :, 0:256])
                    elif t == 4 * n + 2:
                        nc.vector.tensor_copy(exl_acc[:, :, 256:512],
                                              exl[:, :, 0:256])
                    else:
                        nc.vector.tensor_add(exl_acc[:, :, e0:e0 + w],
                                             exl_acc[:, :, e0:e0 + w],
                                             exl[:, :, 0:w])
                if prev_l is not None:
                    emit_pv_l(*prev_l[:6])
                    prev_l[6]()
                prev_l = (t, e0, w, st, sp, exl, acc_l)
            emit_pv_l(*prev_l[:6])
            prev_l[6]()

            # finalize local sums into rows 64/96 and drain them early so the
            # transient bank frees quickly
            sml = psA.tile([128, 2, TCH], F32, tag="qka", name=f"sml{n}")
            for h in range(2):
                nc.tensor.matmul(sml[SMR[2 + h]:SMR[2 + h] + 1, 0, :],
                                 ones_sb[:], exl_acc[:, h, :],
                                 start=True, stop=True,
                                 tile_position=(0, SMR[2 + h]))
            nc.vector.tensor_copy(smsb[64:128, :], sml[64:128, 0, :])
            # full-partition op: custom-DVE reciprocal silently no-ops at
            # base partition 64; rows 0-63 recompute harmlessly
            nc.vector.reciprocal_approx_fast(rcpt[:], smsb[:])

            # ---- global-combine tail: broadcast per-token scales with PE
            # outer products and apply to pv_g (frees the pv_g banks)
            t1s = []
            for h in range(2):
                bcg = psA.tile([128, 2, TCH], F32, tag="qka",
                               name=f"bcg{h}_{n}")[:, 0, :]
                r0 = SMR[h]
                nc.tensor.matmul(bcg[:], onesr_sb[r0:r0 + 1, :],
                                 aglt[r0:r0 + 1, :],
                                 start=True, stop=True,
                                 tile_position=(r0, 0))
                bcgs = combp.tile([128, TCH], BF16, tag="bcs",
                                  name=f"bcgs{h}_{n}", bufs=4)
                nc.vector.tensor_copy(bcgs[:], bcg[:])
                t1 = combp.tile([128, TCH], F32, tag="comb",
                                name=f"t1_{h}_{n}", bufs=4)
                nc.vector.tensor_mul(t1[:], pv_g[:, h, :], bcgs[:])
                t1s.append(t1)
            # full-partition gate scale (rows 64:128 feed the bcl broadcasts
            # in the deferred combine; rows 0:64 recompute identical values)
            nc.vector.tensor_mul(aglt[:], rcpt[:], gstack[:, n, :])

            # ---- local-combine: deferred into the next chunk's global pass
            # so the scale/broadcast chain hides behind fresh PE work
            def local_combine(n=n, pv_l=pv_l, aglt=aglt, t1s=t1s):
                for h in range(2):
                    bcl = psA.tile([128, 2, TCH], F32, tag="qka",
                                   name=f"bcl{h}_{n}")[:, 0, :]
                    r0 = SMR[2 + h]
                    nc.tensor.matmul(bcl[:], onesr_sb[r0:r0 + 1, :],
                                     aglt[r0:r0 + 1, :],
                                     start=True, stop=True,
                                     tile_position=(r0, 0))
                    bcls = combp.tile([128, TCH], BF16, tag="bcs",
                                      name=f"bcls{h}_{n}", bufs=4)
                    nc.vector.tensor_copy(bcls[:], bcl[:])
                    t2 = combp.tile([128, TCH], F32, tag="comb",
                                    name=f"t2_{h}_{n}", bufs=4)
                    ao = aoutp.tile([128, TCH], BF16, tag="aout",
                                    name=f"ao{2 * n + h}")
                    nc.vector.tensor_mul(t2[:], pv_l[:, h, :], bcls[:])
                    nc.vector.tensor_add(ao[:], t1s[h][:], t2[:])
                    # ship the finished [128, 512] block to a2a staging with
                    # ONE strided SWDGE DMA (gpsimd queue): keeps the sync
                    # HWDGE ring free and orders ahead of the trigger below.
                    # token 1024+128c (hi) / 128c (lo) lives in chunk n at
                    # column offset 128j
                    buf = a2ai_hi if n >= 2 else a2ai_lo
                    c0 = (n - 2) * 4 if n >= 2 else n * 4
                    nc.gpsimd.dma_start(
                        out=buf[c0:c0 + 4, h * D:(h + 1) * D, :]
                        .rearrange("j p c -> p j c"),
                        in_=ao[:].rearrange("p (j c) -> p j c", j=4))
                if n == 2:
                    # all-to-all #1: high-token halves (overlaps chunks 1,0)
                    nc.gpsimd.collective_compute(
                        "AllToAll", mybir.AluOpType.bypass,
                        replica_groups=[list(range(NCORES))],
                        ins=[a2ai_hi[:].opt()], outs=[a2ao_hi[:].opt()])

            pend_combine[0] = local_combine

        pend_combine[0]()
        pend_combine[0] = None

        pvlp.release()
        pspv.release()
        psA.release()
        accp.release()
        expp.release()

        # ========= phase 3: all-to-all #2 (low-token halves) =========
        nc.gpsimd.collective_compute(
            "AllToAll", mybir.AluOpType.bypass,
            replica_groups=[list(range(NCORES))],
            ins=[a2ai_lo[:].opt()], outs=[a2ao_lo[:].opt()])
        nc.sync.dma_start(
            out=afull_lo[:, 0:4, :],
            in_=a2ao_lo[0:2].rearrange("r (h p) c -> p (r h) c", h=2))
        for g2 in range(1, 4):
            nc.sync.dma_start(
                out=afull_lo[:, 4 * g2:4 * g2 + 4, :],
                in_=a2ao_lo[2 * g2:2 * g2 + 2]
                .rearrange("r (h p) c -> p (r h) c", h=2))

        pso = tc.alloc_tile_pool(name="pso", bufs=8, space="PSUM")

        # ============ phase 4: o_proj for our token slice ============
        # OUT rows 0-127 = low half-slice, rows 128-255 = high half-slice.
        # hi half first: it only needs all-to-all #1, so the PE works while
        # all-to-all #2 is still in flight. e-outer / k-inner: each 512-col
        # slice drains right after its 16 accumulates so the PSUM copy and
        # OUT store (scalar HWDGE ring; ACT is idle here) overlap the rest.
        for tt, afull in ((1, afull_hi), (0, afull_lo)):
            for ep in range(0, NCH, 2):
                pos = [pso.tile([128, TCH], F32, tag="po",
                                name=f"po_{tt}_{ep + i}") for i in range(2)]
                for k in range(KT):
                    # both e-slices of the pair share one LDWEIGHTS of
                    # afull[:, k, :]
                    for i in range(2):
                        e = ep + i
                        nc.tensor.matmul(
                            pos[i][:],
                            afull[:, k, :],
                            wo_tiles[k // 4][:, k % 4,
                                             e * TCH:(e + 1) * TCH],
                            start=(k == 0), stop=(k == KT - 1))
                for i in range(2):
                    e = ep + i
                    ot = osb.tile([128, TCH], BF16, tag="ot",
                                  name=f"ot{tt}_{e}")
                    nc.vector.tensor_copy(ot[:], pos[i][:])
                    nc.scalar.dma_start(
                        out=OUT[tt * 128:(tt + 1) * 128,
                                e * TCH:(e + 1) * TCH],
                        in_=ot[:])
        pso.release()
        wop.release()
        chunkp.release()
        wqkvp.release()
        combp.release()
        ropet.release()
        work.release()
        osb.release()
        opool.release()
        aoutp.release()
        dram.release()
        const.release()

    nc.compile()
    return nc


def _host_prep(hidden_states, positions, k_global, v_global, w_qkv, w_o,
               w_gate, b_gate):
    """Layout-only host transforms + constant tables -> per-core in_maps."""
    f32 = np.float32
    bf16 = ml_dtypes.bfloat16
    hs = np.asarray(hidden_states, f32)
    pos = np.asarray(positions)
    kg = np.asarray(k_global, f32)
    vg = np.asarray(v_global, f32)
    wqkv = np.asarray(w_qkv, f32)
    wo = np.ascontiguousarray(np.asarray(w_o, f32).astype(bf16))
    wg = np.asarray(w_gate, f32)
    bg = np.asarray(b_gate, f32)

    hst = np.ascontiguousarray(hs.T.astype(bf16))

    half = D // 2
    inv_freq = (THETA ** (-np.arange(half, dtype=f32) / half)).astype(f32)
    ang = pos.astype(f32)[:, None] * inv_freq[None, :]
    cos_t = np.cos(ang).astype(f32).T       # [64, T]
    sin_t = np.sin(ang).astype(f32).T
    csf = np.ascontiguousarray(np.concatenate([cos_t, cos_t], axis=0)).astype(bf16)
    snf = np.ascontiguousarray(np.concatenate([-sin_t, sin_t], axis=0)).astype(bf16)

    p = np.arange(128, dtype=np.int64)[:, None]   # key row within tile
    q = np.arange(128, dtype=np.int64)[None, :]   # query col within block
    # within-block causal triangle for global diagonal tiles (0/1, applied
    # multiplicatively to the exp'd scores); duplicated for the two heads
    maskd = np.where(q >= p, 1.0, 0.0).astype(bf16)
    maskd2 = np.ascontiguousarray(np.concatenate([maskd, maskd], axis=1))
    # canonical local band mask: key row k vs query offset e within a
    # 256-query extent starting at the key tile's base; head-duplicated
    e = np.arange(256, dtype=np.int64)[None, :]
    maskl = np.where((e - p >= 0) & (e - p <= WIN), 1.0, 0.0).astype(bf16)
    maskl2 = np.ascontiguousarray(np.concatenate([maskl, maskl], axis=1))

    ones = np.ones((128, 1), bf16)
    onesr = np.ones((128, 128), bf16)
    idn = np.eye(128, dtype=bf16)

    in_maps = []
    for c in range(NCORES):
        g = c // 2
        wq = wqkv[:, 2 * c * D:(2 * c + 2) * D]
        wk = wqkv[:, HQ * D + g * D:HQ * D + (g + 1) * D]
        wv = wqkv[:, (HQ + HK) * D + g * D:(HQ + HK) * D + (g + 1) * D]
        bgv = np.zeros((33, 1), f32)
        bgv[0, 0] = bg[2 * c]
        bgv[32, 0] = bg[2 * c + 1]
        in_maps.append({
            "HST": hst,
            "WQKV": np.ascontiguousarray(
                np.concatenate([wq, wk, wv], axis=1).astype(bf16)),
            "KGT": np.ascontiguousarray(kg[:, g * D:(g + 1) * D].T.astype(bf16)),
            "VG": np.ascontiguousarray(vg[:, g * D:(g + 1) * D].astype(bf16)),
            "WO": wo,
            "WG": np.ascontiguousarray(wg[:, 2 * c:2 * c + 2].astype(bf16)),
            "BG": bgv,
            "CSF": csf,
            "SNF": snf,
            "ONES": ones,
            "ONESR": onesr,
            "IDN": idn,
            "MASKD2": maskd2,
            "MASKL2": maskl2,
        })
    return in_maps


def kernel(**inputs):
    if "nc" not in _CACHE:
        _CACHE["nc"] = _build()
    nc = _CACHE["nc"]
    in_maps = _host_prep(**inputs)
    res = run_bass_kernel_spmd(nc, in_maps, core_ids=list(range(NCORES)))
    out = np.empty((T, HID), np.float32)
    for c in range(NCORES):
        o = np.asarray(res.results[c]["OUT"]).astype(np.float32)
        out[128 * c:128 * (c + 1)] = o[0:128]
        out[1024 + 128 * c:1024 + 128 * (c + 1)] = o[128:256]
    return out


# revision 42
# speedup vs baseline: 1.0975x; 1.0403x over previous
"""Trainium2 Bass kernel for LoopCoderAttention (sparse_attention).

Head-sharded tensor parallelism over 8 NeuronCores:
  core c owns query heads {2c, 2c+1} and KV head c//2.
All on-device tensors live in transposed [feature, token] layout so every
matmul contracts along the partition dim with zero on-device transposes
(except v, which needs one PE transpose per 128-tile).

v4 notes (on top of the v3 streaming/queue layout):
 - attention scores for BOTH heads of a step land in adjacent PSUM banks
   of one persistent 4-bank "ring" tile, so a single Exp ACTIVATE covers
   both heads (the ACT engine was the steady-state bottleneck at 2 calls
   per step). Global steps ping-pong bank pairs (0,1)/(2,3); the local
   pass packs both heads into ONE bank per key tile (256+256 columns)
   and accumulates pv_l into ring banks 2,3 — freeing enough PSUM for
   the wider score tiles (ring 4 + pv_g 2 + sums 1 + bcast 1 = 8 banks).
 - causal/band masks are applied with one DVE multiply per step against
   head-duplicated mask tables (MASKD2/MASKL2).
 - per-token scale broadcasts (ones outer products) serialize through a
   dedicated 1-bank "bcast" tile instead of stealing score banks.
 - a2a staging is 2 strided SWDGE DMAs per chunk-combine (gpsimd queue),
   immediately followed in the same FIFO by the collective trigger.

o_proj: a 2MB AllToAll reshards attention output from head-sharded to
token-sharded; each core then runs the full 2048-deep contraction for its
256-token slice (the "all-reduce" happens inside the matmul accumulation).
"""
import sys
sys.path.insert(0, '/opt/trn_rl_repo')
import numpy as np
import ml_dtypes
import concourse.bass as bass
import concourse.mybir as mybir
import concourse.tile as tile
from concourse import bacc
from concourse.bass_utils import run_bass_kernel_spmd

T = 2048
HID = 2048
HQ = 16
HK = 4
D = 128
WIN = 64
THETA = 10000.0
SCALE = D ** -0.5
NCORES = 8
TCH = 512                 # t-chunk (matmul free dim)
NCH = T // TCH            # 4 chunks
KT = HID // 128           # 16 k-tiles for 2048-deep contractions
ST = T // 128             # 16 s-tiles
TSL = T // NCORES         # 256-token output slice per core
MASKV = -1e9

F32 = mybir.dt.float32
BF16 = mybir.dt.bfloat16
AF = mybir.ActivationFunctionType

_CACHE = {}


def _build():
    nc = bacc.Bacc("TRN2", target_bir_lowering=False, debug=False,
                   num_devices=NCORES)
    HST = nc.dram_tensor("HST", [HID, T], BF16, kind="ExternalInput").ap()
    WQKV = nc.dram_tensor("WQKV", [HID, 512], BF16, kind="ExternalInput").ap()
    KGT = nc.dram_tensor("KGT", [D, T], BF16, kind="ExternalInput").ap()
    VG = nc.dram_tensor("VG", [T, D], BF16, kind="ExternalInput").ap()
    WO = nc.dram_tensor("WO", [HID, HID], BF16, kind="ExternalInput").ap()
    WG = nc.dram_tensor("WG", [D, 2], BF16, kind="ExternalInput").ap()
    BG = nc.dram_tensor("BG", [33, 1], F32, kind="ExternalInput").ap()
    CSF = nc.dram_tensor("CSF", [128, T], BF16, kind="ExternalInput").ap()
    SNF = nc.dram_tensor("SNF", [128, T], BF16, kind="ExternalInput").ap()
    ONES = nc.dram_tensor("ONES", [128, 1], BF16, kind="ExternalInput").ap()
    ONESR = nc.dram_tensor("ONESR", [128, 128], BF16, kind="ExternalInput").ap()
    IDN = nc.dram_tensor("IDN", [128, 128], BF16, kind="ExternalInput").ap()
    MASKD2 = nc.dram_tensor("MASKD2", [128, 256], BF16,
                            kind="ExternalInput").ap()
    MASKL2 = nc.dram_tensor("MASKL2", [128, 512], BF16,
                            kind="ExternalInput").ap()
    OUT = nc.dram_tensor("OUT", [TSL, HID], BF16, kind="ExternalOutput").ap()

    with tile.TileContext(nc) as tc:
        # pools are a strict stack: creation order is the reverse of the
        # release order at each phase boundary
        const = tc.alloc_tile_pool(name="const", bufs=1)
        dram = tc.alloc_tile_pool(name="dram", bufs=1, space="DRAM")
        aoutp = tc.alloc_tile_pool(name="aoutp", bufs=3)
        opool = tc.alloc_tile_pool(name="opool", bufs=1)
        osb = tc.alloc_tile_pool(name="osb", bufs=3)
        work = tc.alloc_tile_pool(name="work", bufs=1)
        ropet = tc.alloc_tile_pool(name="ropet", bufs=2)
        combp = tc.alloc_tile_pool(name="combp", bufs=2)
        wqkvp = tc.alloc_tile_pool(name="wqkvp", bufs=1)
        chunkp = tc.alloc_tile_pool(name="chunkp", bufs=2)
        hsp = tc.alloc_tile_pool(name="hsp", bufs=1)
        ps1 = tc.alloc_tile_pool(name="ps1", bufs=5, space="PSUM")

        # ---- phase-1 input streaming, interleaved across both HWDGE rings
        # so the first matmul can start right after the ~7us NRT preamble:
        #   scalar ring: wqkv, csf/snf halves, small consts
        #   sync ring:   hs high-token pair (small slabs first), low pair,
        #                then attention constants mid-loop
        wqkv_sb = wqkvp.tile([128, KT, 512], BF16)
        wqkv_view = WQKV.rearrange("(k p) c -> p k c", p=128)
        hst_sb = hsp.tile([128, KT, 4, TCH], BF16)
        hst_view = HST.rearrange("(k p) t -> p k t", p=128)
        nc.scalar.dma_start(out=wqkv_sb[:, 0:2, :], in_=wqkv_view[:, 0:2, :])
        nc.sync.dma_start(out=hst_sb[:, 0:1, 2:4, :],
                          in_=hst_view[:, 0:1, 1024:2048])
        nc.sync.dma_start(out=hst_sb[:, 1:2, 2:4, :],
                          in_=hst_view[:, 1:2, 1024:2048])
        nc.scalar.dma_start(out=wqkv_sb[:, 2:8, :], in_=wqkv_view[:, 2:8, :])
        for kg in range(1, 8):
            nc.sync.dma_start(out=hst_sb[:, 2 * kg:2 * kg + 2, 2:4, :],
                              in_=hst_view[:, 2 * kg:2 * kg + 2, 1024:2048])
        nc.scalar.dma_start(out=wqkv_sb[:, 8:16, :], in_=wqkv_view[:, 8:16, :])
        csf_sb = wqkvp.tile([128, T], BF16)
        snf_sb = wqkvp.tile([128, T], BF16)
        nc.scalar.dma_start(out=csf_sb[:, 1024:2048], in_=CSF[:, 1024:2048])
        nc.scalar.dma_start(out=snf_sb[:, 1024:2048], in_=SNF[:, 1024:2048])
        idn_sb = wqkvp.tile([128, 128], BF16)
        nc.scalar.dma_start(out=idn_sb[:], in_=IDN)
        wg_sb = const.tile([D, 2], BF16)
        nc.scalar.dma_start(out=wg_sb[:], in_=WG)
        bg_sb = const.tile([33, 1], F32)
        nc.scalar.dma_start(out=bg_sb[:], in_=BG)
        # low-token hs pair + low cos/sin: streamed while chunks 3/2 compute
        for kg in range(4):
            nc.sync.dma_start(out=hst_sb[:, 4 * kg:4 * kg + 4, 0:2, :],
                              in_=hst_view[:, 4 * kg:4 * kg + 4, 0:1024])
        nc.scalar.dma_start(out=csf_sb[:, 0:1024], in_=CSF[:, 0:1024])
        nc.scalar.dma_start(out=snf_sb[:, 0:1024], in_=SNF[:, 0:1024])
        # attention-phase constants (emitted mid phase-1 loop, see below)
        kgt_sb = const.tile([D, T], BF16)
        vg_sb = const.tile([128, ST, D], BF16)
        ones_sb = const.tile([128, 1], BF16)
        onesr_sb = const.tile([128, 128], BF16)
        maskd_sb = const.tile([128, 2, 128], BF16)
        maskl_sb = const.tile([128, 2, 256], BF16)

        # ---- persistent work tiles (through attention) ----
        qrot = work.tile([128, 2, T], BF16)
        krot = work.tile([128, T], BF16)
        vcur = work.tile([128, ST, D], BF16)   # current v in [s, d] tiles
        # gates staged at the partitions where the softmax-sum rows land:
        # row 0 = g_h0, 32 = g_h1, 64 = 1-g_h0, 96 = 1-g_h1 (per chunk n)
        gstack = work.tile([128, NCH, TCH], F32)
        SMR = (0, 32, 64, 96)
        # negated gate bias for the exp-based sigmoid of the flushed chunk
        bgn_sb = work.tile([33, 1], F32)
        nc.vector.tensor_scalar(bgn_sb[:], bg_sb[:], -1.0, 0.0,
                                mybir.AluOpType.mult, mybir.AluOpType.add)

        # tiny dummy collective fired at kernel start: absorbs the cc-stream
        # init barrier and the ~11.5us first-trigger penalty under phase 1,
        # so the real all-to-alls start promptly
        dmy_i = dram.tile([NCORES, 16], BF16)
        dmy_o = dram.tile([NCORES, NCORES, 16], BF16)
        a2ai_hi = dram.tile([NCORES, 2 * D, TSL // 2], BF16)
        a2ao_hi = dram.tile([NCORES, 2 * D, TSL // 2], BF16)
        a2ai_lo = dram.tile([NCORES, 2 * D, TSL // 2], BF16)
        a2ao_lo = dram.tile([NCORES, 2 * D, TSL // 2], BF16)

        def rope_chunk(dst_full, src, n):
            """dst_full[:, n*TCH:...] = neox-rope of chunk tile src [128, TCH].

            rot = src * [cos;cos] + rot90(src) * [-sin;sin], where rot90 swaps
            the two 64-partition halves (built with two SBUF->SBUF DMAs since
            DVE ops require matching base partitions).
            """
            sl = bass.ds(n * TCH, TCH)
            sr = ropet.tile([128, TCH], BF16, tag="ropesr", name=f"sr{n}")
            # scalar HWDGE ring: chains right behind the ACT copy that
            # produced src, and keeps the sync ring free for bulk loads
            nc.scalar.dma_start(out=sr[0:64, :], in_=src[64:128, :])
            nc.scalar.dma_start(out=sr[64:128, :], in_=src[0:64, :])
            ta = ropet.tile([128, TCH], BF16, tag="ropetmp", name=f"ra{n}")
            tb = ropet.tile([128, TCH], BF16, tag="ropetmp", name=f"rb{n}")
            nc.vector.tensor_mul(ta[:], src[:], csf_sb[:, sl])
            nc.vector.tensor_mul(tb[:], sr[:], snf_sb[:, sl])
            nc.vector.tensor_add(dst_full[:, sl], ta[:], tb[:])

        nc.gpsimd.collective_compute(
            "AllGather", mybir.AluOpType.bypass,
            replica_groups=[list(range(NCORES))],
            ins=[dmy_i[:].opt()], outs=[dmy_o[:].opt()])

        # ================= phase 1: qkvT = wqkv^T @ hsT =================
        # chunks descend: high-token pair (3,2) first so attention on chunk 3
        # can begin while the low pair computes
        pending_small = []
        for n in reversed(range(NCH)):
            pss = [ps1.tile([128, TCH], F32, tag="ps1t", name=f"ps1_{n}_{m}")
                   for m in range(4)]
            for k in range(KT):
                for m in range(4):
                    nc.tensor.matmul(pss[m][:],
                                     wqkv_sb[:, k, m * 128:(m + 1) * 128],
                                     hst_sb[:, k, n, :],
                                     start=(k == 0), stop=(k == KT - 1))
            if pending_small:
                pending_small.pop(0)()
            if n == 1:
                # attention constants: by now the hs low pair is streamed, so
                # these ride the sync ring without starving phase 1
                nc.sync.dma_start(out=kgt_sb[:], in_=KGT)
                nc.sync.dma_start(out=vg_sb[:],
                                  in_=VG.rearrange("(s p) d -> p s d", p=128))
                nc.sync.dma_start(out=ones_sb[:], in_=ONES)
                nc.sync.dma_start(out=onesr_sb[:], in_=ONESR)
                nc.sync.dma_start(
                    out=maskd_sb[:],
                    in_=MASKD2.rearrange("p (h c) -> p h c", h=2))
                nc.sync.dma_start(
                    out=maskl_sb[:],
                    in_=MASKL2.rearrange("p (h c) -> p h c", h=2))
            sl = bass.ds(n * TCH, TCH)
            q0c = chunkp.tile([128, TCH], BF16, tag="q0c")
            q1c = chunkp.tile([128, TCH], BF16, tag="q1c")
            kc = chunkp.tile([128, TCH], BF16, tag="kc")
            vc = chunkp.tile([128, TCH], BF16, tag="vc")
            nc.scalar.activation(q0c[:], pss[0][:], AF.Copy)
            nc.scalar.activation(q1c[:], pss[1][:], AF.Copy)
            nc.scalar.activation(kc[:], pss[2][:], AF.Copy)
            nc.vector.tensor_copy(vc[:], pss[3][:])

            rope_chunk(qrot[:, 0, :], q0c, n)
            rope_chunk(qrot[:, 1, :], q1c, n)
            rope_chunk(krot, kc, n)

            def small_ops(n=n, vc=vc, sl=sl, attn=False):
                # v transposes + gates for chunk n: emitted one chunk later so
                # the PE stream never waits on the DVE rope/copy latency.
                # attn=True -> running inside the attention phase (chunk 0's
                # deferred ops): draw PSUM from the attention score pool
                for j in range(4):
                    s = 4 * n + j
                    if attn:
                        pt = psA.tile([128, 128], BF16, tag="qka",
                                      name=f"pt{s}")
                    else:
                        pt = ps1.tile([128, 128], BF16, tag="ps1g",
                                      name=f"pt{s}", bufs=2)
                    nc.tensor.transpose(pt[:], vc[:, j * 128:(j + 1) * 128],
                                        idn_sb[:])
                    nc.vector.tensor_copy(vcur[:, s, :], pt[:])
                # gates for both heads in one PSUM tile: h0 -> partition 0,
                # h1 -> partition 32 (col-tiled), one Sigmoid serves both
                if attn:
                    gp = psA.tile([33, TCH], F32, tag="qka", name=f"gp{n}")
                else:
                    gp = ps1.tile([33, TCH], F32, tag="ps1s", name=f"gp{n}",
                                  bufs=1)
                nc.tensor.matmul(gp[0:1, :], wg_sb[:, 0:1], qrot[:, 0, sl],
                                 start=True, stop=True,
                                 tile_position=(0, 0))
                nc.tensor.matmul(gp[32:33, :], wg_sb[:, 1:2], qrot[:, 1, sl],
                                 start=True, stop=True,
                                 tile_position=(0, 32))
                if attn:
                    # mid-attention: a Sigmoid here would force two ACT
                    # table-set switches (~2.7us each, on every core).
                    # Compute it with the already-loaded Exp set instead:
                    # e = exp(-(z+b)); g = 1/(1+e); 1-g = e*g
                    ge = chunkp.tile([33, TCH], F32, tag="ge", name=f"ge{n}", bufs=1)
                    nc.scalar.activation(ge[:], gp[:], AF.Exp, scale=-1.0,
                                         bias=bgn_sb[:])
                    gt = chunkp.tile([33, TCH], F32, tag="gt", name=f"gt{n}", bufs=1)
                    nc.vector.tensor_scalar(gt[:], ge[:], 1.0, 1.0,
                                            mybir.AluOpType.mult,
                                            mybir.AluOpType.add)
                    gg = chunkp.tile([33, TCH], F32, tag="gg", name=f"gg{n}", bufs=1)
                    nc.vector.reciprocal_approx_fast(gg[:], gt[:])
                    nc.scalar.dma_start(out=gstack[0:33, n, :], in_=gg[:])
                    g1 = chunkp.tile([33, TCH], F32, tag="g1", name=f"g1{n}", bufs=1)
                    nc.vector.tensor_mul(g1[:], ge[:], gg[:])
                    nc.scalar.dma_start(out=gstack[64:97, n, :], in_=g1[:])
                else:
                    gst = chunkp.tile([33, TCH], F32, tag="gst",
                                      name=f"gst{n}")
                    nc.scalar.activation(gst[:], gp[:], AF.Sigmoid,
                                         bias=bg_sb[:])
                    nc.scalar.dma_start(out=gstack[0:33, n, :], in_=gst[:])
                    nc.scalar.dma_start(out=gstack[64:97, n, :], in_=gst[:])
                    nc.vector.tensor_scalar(gstack[64:97, n, :],
                                            gstack[64:97, n, :], -1.0, 1.0,
                                            mybir.AluOpType.mult,
                                            mybir.AluOpType.add)

            pending_small.append(small_ops)

        # chunk 0's small_ops are NOT flushed here: they run at the start of
        # the attention phase (drawing PSUM from the attention pools), so
        # attention QKs don't sit behind the phase-1 pool-release barrier.
        # chunkp/wqkvp stay alive through attention for vc(chunk0)/idn.
        ps1.release()
        hsp.release()

        # w_o prefetch: emitted now so the 8MB streams in during attention,
        # well before anything else joins the sync queue
        wop = tc.alloc_tile_pool(name="wop", bufs=4)
        wo_tiles = []
        wo_view = WO.rearrange("(k p) c -> p k c", p=128)
        for kq in range(4):
            wo_t = wop.tile([128, 4, HID], BF16, tag="wo", name=f"wo{kq}")
            nc.sync.dma_start(out=wo_t[:],
                              in_=wo_view[:, 4 * kq:4 * kq + 4, :])
            wo_tiles.append(wo_t)

        afull_hi = opool.tile([128, KT, TSL // 2], BF16)
        afull_lo = opool.tile([128, KT, TSL // 2], BF16)

        expp = tc.alloc_tile_pool(name="expp", bufs=4)
        accp = tc.alloc_tile_pool(name="accp", bufs=2)
        psA = tc.alloc_tile_pool(name="psA", bufs=2, space="PSUM")
        pspv = tc.alloc_tile_pool(name="pspv", bufs=1, space="PSUM")
        pvlp = tc.alloc_tile_pool(name="pvlp", bufs=1, space="PSUM")

        # chunk 0's deferred v-transposes + gates are flushed INSIDE the
        # first attention chunk (after its global pass) — see the loop

        # ============ phase 2: attention (global + local) ============
        # chunks descend so the high-token half finishes first and its
        # all-to-all overlaps the low-token half's compute.
        #
        # PSUM budget (8 banks): score pair tiles 2x2 + pv_g 2 + pv_l 2.
        # Both heads' QK scores of a step land in one fp32 pair tile and a
        # single Exp ACTIVATE covers both heads. Softmax sums never touch
        # the PE inside the step loop: the DVE accumulates the exp tiles
        # elementwise (off the critical path) and one ones-matmul per
        # pass/chunk reduces the accumulator across partitions into rows
        # 0/32 (global) and 64/96 (local) of a transient score-pool bank.
        pend_combine = [None]

        for n in reversed(range(NCH)):
            sl = bass.ds(n * TCH, TCH)
            pv_g = pspv.tile([128, 2, TCH], F32, tag="pv", name=f"pvg{n}")
            pv_l = pvlp.tile([128, 2, TCH], F32, tag="pvl", name=f"pvl{n}")
            exg_acc = accp.tile([128, 2, TCH], BF16, tag="acc",
                                name=f"exga{n}")
            exl_acc = accp.tile([128, 2, TCH], BF16, tag="accl",
                                name=f"exla{n}")
            smsb = combp.tile([128, TCH], F32, tag="smsb", name=f"smsb{n}")
            rcpt = combp.tile([128, TCH], F32, tag="rcpt", name=f"rcpt{n}")
            aglt = combp.tile([128, TCH], BF16, tag="aglt", name=f"aglt{n}")

            # ---- global pass over cached KV (both heads share k/v tiles);
            # diagonal tiles stream only the causally-live query extent
            ns = 4 * n + 4

            # software pipeline: emit step s's QK+exp, then step s-1's PV
            # (whose exp finished during this step's QKs) so the PE never
            # waits on the ACT engine inside a step
            def emit_pv_g(s, jo, ex):
                for h in range(2):
                    nc.tensor.matmul(pv_g[:, h, jo:], vg_sb[:, s, :],
                                     ex[:, h, jo:],
                                     start=(s == 0), stop=(s == ns - 1))

            hook_s = 2 if ns == 4 else 4
            prev_g = None
            for s in range(ns):
                if s == hook_s and pend_combine[0] is not None:
                    pend_combine[0]()
                    pend_combine[0] = None
                jo = max(0, (s - 4 * n) * 128)
                mv = bass.ds(n * TCH + jo, TCH - jo)
                qk = psA.tile([128, 2, TCH], F32, tag="qka",
                              name=f"qka_{n}_{s}")
                for h in range(2):
                    nc.tensor.matmul(qk[:, h, jo:],
                                     kgt_sb[:, s * 128:(s + 1) * 128],
                                     qrot[:, h, mv], start=True, stop=True)
                ex = expp.tile([128, 2, TCH], BF16, tag="ex",
                               name=f"exg_{n}_{s}")
                nc.scalar.activation(ex[:, :, jo:], qk[:, :, jo:],
                                     AF.Exp, scale=SCALE)
                if s >= 4 * n:
                    # multiplicative 0/1 causal mask on the in-block
                    # triangle, both heads at once
                    nc.vector.tensor_mul(ex[:, :, jo:jo + 128],
                                         ex[:, :, jo:jo + 128],
                                         maskd_sb[:])
                # softmax-sum partial: elementwise accumulate over key tiles
                # (s=0 always has jo=0, so the copy initializes fully).
                # Deferred one step so it never sits ahead of the next
                # step's mask multiply in the DVE FIFO.
                def acc_g(s=s, jo=jo, ex=ex):
                    if s == 0:
                        nc.vector.tensor_copy(exg_acc[:], ex[:])
                    else:
                        nc.vector.tensor_add(exg_acc[:, :, jo:],
                                             exg_acc[:, :, jo:],
                                             ex[:, :, jo:])
                if prev_g is not None:
                    emit_pv_g(*prev_g[:3])
                    prev_g[3]()
                prev_g = (s, jo, ex, acc_g)
            emit_pv_g(*prev_g[:3])
            prev_g[3]()

            # finalize global sums: one partition-reduce matmul per head into
            # rows 0/32 of a transient score-pool bank
            smg = psA.tile([128, 2, TCH], F32, tag="qka", name=f"smg{n}")
            for h in range(2):
                nc.tensor.matmul(smg[SMR[h]:SMR[h] + 1, 0, :], ones_sb[:],
                                 exg_acc[:, h, :], start=True, stop=True,
                                 tile_position=(0, SMR[h]))

            if pending_small:
                # chunk 0's deferred v-transposes + gates: by now their
                # inputs (vc/qrot of chunk 0) have long drained, and chunk
                # 3's global QKs didn't have to queue behind them
                pending_small.pop(0)(attn=True)

            if n == 0:
                # gather all-to-all #1 results now: the collective is done (or
                # nearly so), so this never head-blocks the sync DMA queue
                nc.sync.dma_start(
                    out=afull_hi[:, 0:4, :],
                    in_=a2ao_hi[0:2].rearrange("r (h p) c -> p (r h) c",
                                               h=2))
                for g2 in range(1, 4):
                    nc.sync.dma_start(
                        out=afull_hi[:, 4 * g2:4 * g2 + 4, :],
                        in_=a2ao_hi[2 * g2:2 * g2 + 2]
                        .rearrange("r (h p) c -> p (r h) c", h=2))

            # ---- global-combine DVE half: drain g-sum rows, reciprocal,
            # scale by gate; overlaps the local pass below
            nc.vector.tensor_copy(smsb[0:64, :], smg[0:64, 0, :])
            nc.vector.reciprocal_approx_fast(rcpt[0:64, :], smsb[0:64, :])
            nc.vector.tensor_mul(aglt[0:64, :], rcpt[0:64, :],
                                 gstack[0:64, n, :])

            # ---- local sliding-window pass over current KV: both heads of a
            # key tile share one fp32 bank ([2, 256] = 512 fp32)
            def emit_pv_l(t, e0, w, st, sp, exl):
                osl = bass.ds(e0, w)
                for h in range(2):
                    nc.tensor.matmul(pv_l[:, h, osl], vcur[:, t, :],
                                     exl[:, h, 0:w],
                                     start=st, stop=sp,
                                     skip_group_check=True)

            # one matmul per key tile over a padded 256-query extent with a
            # single shift-invariant band mask. Tiles 4n and 4n+2 are emitted
            # first with start=True: their extents exactly partition [0,512)
            # so the remaining tiles accumulate with start=False.
            # start=True only on the first tile: it row-clears the whole
            # bank, so tile 4n+2's disjoint region write-if-cleans correctly
            # and the overlapping odd tiles accumulate
            lt = [(4 * n, 0, 256, 0, True, False),
                  (4 * n + 2, 256, 256, 0, False, False),
                  (4 * n - 1, 0, 64, 128, False, True),
                  (4 * n + 1, 128, 192, 0, False, True),
                  (4 * n + 3, 384, 128, 0, False, True)]
            prev_l = None
            for (t, e0, w, m0, st, sp) in lt:
                if t < 0:
                    continue
                qsl = bass.ds(n * TCH + e0, w)
                qka_t = psA.tile([128, 2, TCH], F32, tag="qka",
                                 name=f"qkl_{n}_{t}")
                qkl = qka_t[:, 0, :].rearrange("p (h c) -> p h c", h=2)
                for h in range(2):
                    nc.tensor.matmul(qkl[:, h, 0:w],
                                     krot[:, t * 128:(t + 1) * 128],
                                     qrot[:, h, qsl],
                                     start=True, stop=True)
                exl = expp.tile([128, 2, 256], BF16, tag="exl",
                                name=f"exl_{n}_{t}")
                nc.scalar.activation(exl[:, :, 0:w], qkl[:, :, 0:w],
                                     AF.Exp, scale=SCALE)
                nc.vector.tensor_mul(exl[:, :, 0:w], exl[:, :, 0:w],
                                     maskl_sb[:, :, m0:m0 + w])
                # local softmax-sum partials: tiles 4n / 4n+2 initialize the
                # two disjoint halves; the overlapping tiles accumulate.
                # Deferred one tile (DVE FIFO: keep masks ahead of accs).
                def acc_l(t=t, e0=e0, w=w, exl=exl):
                    if t == 4 * n:
                        nc.vector.tensor_copy(exl_acc[:, :, 0:256],
                                              exl[:, :, 0:256])
                    elif t == 4 * n + 2:
                        nc.vector.tensor_copy(exl_acc[:, :, 256:512],
                                              exl[:, :, 0:256])
                    else:
                        nc.vector.tensor_add(exl_acc[:, :, e0:e0 + w],
                                             exl_acc[:, :, e0:e0 + w],
                                             exl[:, :, 0:w])
                if prev_l is not None:
                    emit_pv_l(*prev_l[:6])
                    prev_l[6]()
                prev_l = (t, e0, w, st, sp, exl, acc_l)
            emit_pv_l(*prev_l[:6])
            prev_l[6]()

            # finalize local sums into rows 64/96 and drain them early so the
            # transient bank frees quickly
            sml = psA.tile([128, 2, TCH], F32, tag="qka", name=f"sml{n}")
            for h in range(2):
                nc.tensor.matmul(sml[SMR[2 + h]:SMR[2 + h] + 1, 0, :],
                                 ones_sb[:], exl_acc[:, h, :],
                                 start=True, stop=True,
                                 tile_position=(0, SMR[2 + h]))
            nc.vector.tensor_copy(smsb[64:128, :], sml[64:128, 0, :])
            # full-partition op: custom-DVE reciprocal silently no-ops at
            # base partition 64; rows 0-63 recompute harmlessly
            nc.vector.reciprocal_approx_fast(rcpt[:], smsb[:])

            # ---- global-combine tail: broadcast per-token scales with PE
            # outer products and apply to pv_g (frees the pv_g banks)
            t1s = []
            for h in range(2):
                bcg = psA.tile([128, 2, TCH], F32, tag="qka",
                               name=f"bcg{h}_{n}")[:, 0, :]
                r0 = SMR[h]
                nc.tensor.matmul(bcg[:], onesr_sb[r0:r0 + 1, :],
                                 aglt[r0:r0 + 1, :],
                                 start=True, stop=True,
                                 tile_position=(r0, 0))
                bcgs = combp.tile([128, TCH], BF16, tag="bcs",
                                  name=f"bcgs{h}_{n}", bufs=4)
                nc.vector.tensor_copy(bcgs[:], bcg[:])
                t1 = combp.tile([128, TCH], F32, tag="comb",
                                name=f"t1_{h}_{n}", bufs=4)
                nc.vector.tensor_mul(t1[:], pv_g[:, h, :], bcgs[:])
                t1s.append(t1)
            # full-partition gate scale (rows 64:128 feed the bcl broadcasts
            # in the deferred combine; rows 0:64 recompute identical values)
            nc.vector.tensor_mul(aglt[:], rcpt[:], gstack[:, n, :])

            # ---- local-combine: deferred into the next chunk's global pass
            # so the scale/broadcast chain hides behind fresh PE work
            def local_combine(n=n, pv_l=pv_l, aglt=aglt, t1s=t1s):
                for h in range(2):
                    bcl = psA.tile([128, 2, TCH], F32, tag="qka",
                                   name=f"bcl{h}_{n}")[:, 0, :]
                    r0 = SMR[2 + h]
                    nc.tensor.matmul(bcl[:], onesr_sb[r0:r0 + 1, :],
                                     aglt[r0:r0 + 1, :],
                                     start=True, stop=True,
                                     tile_position=(r0, 0))
                    bcls = combp.tile([128, TCH], BF16, tag="bcs",
                                      name=f"bcls{h}_{n}", bufs=4)
                    nc.vector.tensor_copy(bcls[:], bcl[:])
                    t2 = combp.tile([128, TCH], F32, tag="comb",
                                    name=f"t2_{h}_{n}", bufs=4)
                    ao = aoutp.tile([128, TCH], BF16, tag="aout",
                                    name=f"ao{2 * n + h}")
                    nc.vector.tensor_mul(t2[:], pv_l[:, h, :], bcls[:])
                    nc.vector.tensor_add(ao[:], t1s[h][:], t2[:])
                    # ship the finished [128, 512] block to a2a staging with
                    # ONE strided SWDGE DMA (gpsimd queue): keeps the sync
                    # HWDGE ring free and orders ahead of the trigger below.
                    # token 1024+128c (hi) / 128c (lo) lives in chunk n at
                    # column offset 128j
                    buf = a2ai_hi if n >= 2 else a2ai_lo
                    c0 = (n - 2) * 4 if n >= 2 else n * 4
                    nc.gpsimd.dma_start(
                        out=buf[c0:c0 + 4, h * D:(h + 1) * D, :]
                        .rearrange("j p c -> p j c"),
                        in_=ao[:].rearrange("p (j c) -> p j c", j=4))
                if n == 2:
                    # all-to-all #1: high-token halves (overlaps chunks 1,0)
                    nc.gpsimd.collective_compute(
                        "AllToAll", mybir.AluOpType.bypass,
                        replica_groups=[list(range(NCORES))],
                        ins=[a2ai_hi[:].opt()], outs=[a2ao_hi[:].opt()])

            pend_combine[0] = local_combine

        pend_combine[0]()
        pend_combine[0] = None

        pvlp.release()
        pspv.release()
        psA.release()
        accp.release()
        expp.release()

        # ========= phase 3: all-to-all #2 (low-token halves) =========
        nc.gpsimd.collective_compute(
            "AllToAll", mybir.AluOpType.bypass,
            replica_groups=[list(range(NCORES))],
            ins=[a2ai_lo[:].opt()], outs=[a2ao_lo[:].opt()])
        nc.sync.dma_start(
            out=afull_lo[:, 0:4, :],
            in_=a2ao_lo[0:2].rearrange("r (h p) c -> p (r h) c", h=2))
        for g2 in range(1, 4):
            nc.sync.dma_start(
                out=afull_lo[:, 4 * g2:4 * g2 + 4, :],
                in_=a2ao_lo[2 * g2:2 * g2 + 2]
                .rearrange("r (h p) c -> p (r h) c", h=2))

        pso = tc.alloc_tile_pool(name="pso", bufs=8, space="PSUM")

        # ============ phase 4: o_proj for our token slice ============
        # OUT rows 0-127 = low half-slice, rows 128-255 = high half-slice.
        # hi half first: it only needs all-to-all #1, so the PE works while
        # all-to-all #2 is still in flight. e-outer / k-inner: each 512-col
        # slice drains right after its 16 accumulates so the PSUM copy and
        # OUT store (scalar HWDGE ring; ACT is idle here) overlap the rest.
        for tt, afull in ((1, afull_hi), (0, afull_lo)):
            for ep in range(0, NCH, 2):
                pos = [pso.tile([128, TCH], F32, tag="po",
                                name=f"po_{tt}_{ep + i}") for i in range(2)]
                for k in range(KT):
                    # both e-slices of the pair share one LDWEIGHTS of
                    # afull[:, k, :]
                    for i in range(2):
                        e = ep + i
                        nc.tensor.matmul(
                            pos[i][:],
                            afull[:, k, :],
                            wo_tiles[k // 4][:, k % 4,
                                             e * TCH:(e + 1) * TCH],
                            start=(k == 0), stop=(k == KT - 1))
                for i in range(2):
                    e = ep + i
                    ot = osb.tile([128, TCH], BF16, tag="ot",
                                  name=f"ot{tt}_{e}")
                    nc.vector.tensor_copy(ot[:], pos[i][:])
                    nc.scalar.dma_start(
                        out=OUT[tt * 128:(tt + 1) * 128,
                                e * TCH:(e + 1) * TCH],
                        in_=ot[:])
        pso.release()
        wop.release()
        chunkp.release()
        wqkvp.release()
        combp.release()
        ropet.release()
        work.release()
        osb.release()
        opool.release()
        aoutp.release()
        dram.release()
        const.release()

    nc.compile()
    return nc


def _host_prep(hidden_states, positions, k_global, v_global, w_qkv, w_o,
               w_gate, b_gate):
    """Layout-only host transforms + constant tables -> per-core in_maps."""
    f32 = np.float32
    bf16 = ml_dtypes.bfloat16
    hs = np.asarray(hidden_states, f32)
    pos = np.asarray(positions)
    kg = np.asarray(k_global, f32)
    vg = np.asarray(v_global, f32)
    wqkv = np.asarray(w_qkv, f32)
    wo = np.ascontiguousarray(np.asarray(w_o, f32).astype(bf16))
    wg = np.asarray(w_gate, f32)
    bg = np.asarray(b_gate, f32)

    hst = np.ascontiguousarray(hs.T.astype(bf16))

    half = D // 2
    inv_freq = (THETA ** (-np.arange(half, dtype=f32) / half)).astype(f32)
    ang = pos.astype(f32)[:, None] * inv_freq[None, :]
    cos_t = np.cos(ang).astype(f32).T       # [64, T]
    sin_t = np.sin(ang).astype(f32).T
    csf = np.ascontiguousarray(np.concatenate([cos_t, cos_t], axis=0)).astype(bf16)
    snf = np.ascontiguousarray(np.concatenate([-sin_t, sin_t], axis=0)).astype(bf16)

    p = np.arange(128, dtype=np.int64)[:, None]   # key row within tile
    q = np.arange(128, dtype=np.int64)[None, :]   # query col within block
    # within-block causal triangle for global diagonal tiles (0/1, applied
    # multiplicatively to the exp'd scores); duplicated for the two heads
    maskd = np.where(q >= p, 1.0, 0.0).astype(bf16)
    maskd2 = np.ascontiguousarray(np.concatenate([maskd, maskd], axis=1))
    # canonical local band mask: key row k vs query offset e within a
    # 256-query extent starting at the key tile's base; head-duplicated
    e = np.arange(256, dtype=np.int64)[None, :]
    maskl = np.where((e - p >= 0) & (e - p <= WIN), 1.0, 0.0).astype(bf16)
    maskl2 = np.ascontiguousarray(np.concatenate([maskl, maskl], axis=1))

    ones = np.ones((128, 1), bf16)
    onesr = np.ones((128, 128), bf16)
    idn = np.eye(128, dtype=bf16)

    in_maps = []
    for c in range(NCORES):
        g = c // 2
        wq = wqkv[:, 2 * c * D:(2 * c + 2) * D]
        wk = wqkv[:, HQ * D + g * D:HQ * D + (g + 1) * D]
        wv = wqkv[:, (HQ + HK) * D + g * D:(HQ + HK) * D + (g + 1) * D]
        bgv = np.zeros((33, 1), f32)
        bgv[0, 0] = bg[2 * c]
        bgv[32, 0] = bg[2 * c + 1]
        in_maps.append({
            "HST": hst,
            "WQKV": np.ascontiguousarray(
                np.concatenate([wq, wk, wv], axis=1).astype(bf16)),
            "KGT": np.ascontiguousarray(kg[:, g * D:(g + 1) * D].T.astype(bf16)),
            "VG": np.ascontiguousarray(vg[:, g * D:(g + 1) * D].astype(bf16)),
            "WO": wo,
            "WG": np.ascontiguousarray(wg[:, 2 * c:2 * c + 2].astype(bf16)),
            "BG": bgv,
            "CSF": csf,
            "SNF": snf,
            "ONES": ones,
            "ONESR": onesr,
            "IDN": idn,
            "MASKD2": maskd2,
            "MASKL2": maskl2,
        })
    return in_maps


def kernel(**inputs):
    if "nc" not in _CACHE:
        _CACHE["nc"] = _build()
    nc = _CACHE["nc"]
    in_maps = _host_prep(**inputs)
    res = run_bass_kernel_spmd(nc, in_maps, core_ids=list(range(NCORES)))
    out = np.empty((T, HID), np.float32)
    for c in range(NCORES):
        o = np.asarray(res.results[c]["OUT"]).astype(np.float32)
        out[128 * c:128 * (c + 1)] = o[0:128]
        out[1024 + 128 * c:1024 + 128 * (c + 1)] = o[128:256]
    return out
